# revision 32
# baseline (speedup 1.0000x reference)
"""Adaptive MSE loss (min over shifts) on 8 TRN2 NeuronCores.

Full inputs: input [16,64,8192] f32, target [16,64,10240] f32.
Data-parallel over batch B: 2 batches/core -> bc=128 rows on partitions.

Per core:
  P[m,u]    = sum_a sum_bc (-2*inp_bf)[bc,128a+m] * tgt_bf[bc,128a+u]   (PE, bf16)
  -2corr[s] = sum_m P[m, m+s]          (dense DRAM write + stride-2177 read skew,
                                        then one-hot fp32 matmuls -> [17,128])
  winsum[s] = CW[q] - Hx[q,r] + Hx[q+64,r]   (s=128q+r; block prefix sums of t2)
  t2[u]     = sum_bc tgt[bc,u]^2       (ones-matmul, bf16)
  inp_sq    = sum inp^2                (ACT square+accum, folded into CW matmul)
  loss*n    = inp_sq + winsum - 2corr  -> AllReduce [17,128] -> argmin on-device.

Inputs are loaded in chunks with casts/squares pipelined per chunk so the
PE starts ~15us in; a tiny warmup collective at t=0 absorbs ncfw init.
"""

import sys
import numpy as np

sys.path.insert(0, "/opt/trn_rl_repo")

from concourse import bass, mybir  # noqa: E402
from concourse.ap import AP  # noqa: E402

F32 = mybir.dt.float32
BF16 = mybir.dt.bfloat16
I32 = mybir.dt.int32

B, C, LIN, LTGT = 16, 64, 8192, 10240
NCORES = 8
BC = (B // NCORES) * C            # 128 rows per core
S = LTGT - LIN + 1                # 2049 shifts
AW = LIN // 128                   # 64 contraction chunks
PW = 2176                         # P width (17*128)
NQ = 17                           # shift blocks (s = 128q+r)
NBLK = LTGT // 128                # 80 t2 blocks
NTOT = float(B * C * LIN)         # 8388608
SKST = PW + 1                     # 2177: skew read row stride (write is dense)

ICH = 2                           # input DMA chunks (4096 cols each)
TCH = 4                           # target DMA chunks (2560 cols each)
ICW = LIN // ICH
TCW = LTGT // TCH
T2MM_AT = 40                      # corr-loop index where t2 matmuls interleave
NT2 = LTGT // 512                 # 20 t2 slabs


def build_bass():
    nc = bass.Bass(num_devices=NCORES)

    inp_ext = nc.declare_dram_parameter("input", [BC, LIN], F32, isOutput=False)
    tgt_ext = nc.declare_dram_parameter("target", [BC, LTGT], F32, isOutput=False)
    out_ext = nc.declare_dram_parameter("out", [1, 2], F32, isOutput=True)

    t2_dram = nc.dram_tensor("t2_dram", [NBLK * 128], F32)
    pskew_dram = nc.dram_tensor("pskew_dram", [128 * PW], F32)
    lossp_dram = nc.dram_tensor("lossp_dram", [NQ, 128], F32)
    lossr_dram = nc.dram_tensor("lossr_dram", [NQ, 128], F32, addr_space="Shared")
    ccw_in_dram = nc.dram_tensor("ccw_in_dram", [128], F32)
    ccw_out_dram = nc.dram_tensor("ccw_out_dram", [128], F32, addr_space="Shared")

    # SBUF
    inp32 = nc.alloc_sbuf_tensor("inp32", [BC, LIN], F32)
    tgt32 = nc.alloc_sbuf_tensor("tgt32", [BC, LTGT], F32)
    inpbf = nc.alloc_sbuf_tensor("inpbf", [BC, LIN], BF16)
    tgtbf = nc.alloc_sbuf_tensor("tgtbf", [BC, LTGT], BF16)
    sqbf = nc.alloc_sbuf_tensor("sqbf", [BC, LTGT], BF16)
    sqscr = nc.alloc_sbuf_tensor("sqscr", [BC, 2048], BF16)
    sqacc = nc.alloc_sbuf_tensor("sqacc", [BC, 8], F32)
    colsum1 = nc.alloc_sbuf_tensor("colsum1", [BC, 1], F32)
    t2flat = nc.alloc_sbuf_tensor("t2flat", [1, LTGT], F32)
    psb_off = nc.sbuf_base
    psb = nc.alloc_sbuf_tensor("psb", [128, PW], F32)
    # skewsb aliases psb: psb is dead once its DMA-out completes, and the
    # skew-read DMA that fills skewsb is sem-ordered after that DMA-out.
    skewsb = nc.alloc_sbuf_tensor_at("skewsb", [128, S], F32, offset=psb_off)
    t2sb = nc.alloc_sbuf_tensor("t2sb", [128, 128], F32)
    t2sb2 = nc.alloc_sbuf_tensor("t2sb2", [128, 128], F32)
    hx = nc.alloc_sbuf_tensor("hx", [128, 128], F32)
    htop = nc.alloc_sbuf_tensor("htop", [NQ, 128], F32)
    loss_sb = nc.alloc_sbuf_tensor("loss_sb", [NQ, 128], F32)
    lossr_sb = nc.alloc_sbuf_tensor("lossr_sb", [NQ, 128], F32)
    cw_sb = nc.alloc_sbuf_tensor("cw_sb", [NQ, 1], F32)
    b_bf = nc.alloc_sbuf_tensor("b_bf", [128, 1], BF16)
    colsum_bf = nc.alloc_sbuf_tensor("colsum_bf", [128, 1], BF16)
    iota_i = nc.alloc_sbuf_tensor("iota_i", [NQ, 128], I32)
    iota_f = nc.alloc_sbuf_tensor("iota_f", [NQ, 128], F32)
    maskb = nc.alloc_sbuf_tensor("maskb", [128, NQ], BF16)
    ones17 = nc.alloc_sbuf_tensor("ones17", [128, NQ], BF16)
    onecol = nc.alloc_sbuf_tensor("onecol", [128, 1], BF16)
    onehot = nc.alloc_sbuf_tensor("onehot", [128, 2 * NQ - 1], F32)
    maskadd = nc.alloc_sbuf_tensor("maskadd", [NQ, 128], F32)
    rs_sb = nc.alloc_sbuf_tensor("rs_sb", [NQ, 2], F32)
    rst_sb = nc.alloc_sbuf_tensor("rst_sb", [1, 2 * NQ], F32)
    u_sb = nc.alloc_sbuf_tensor("u_sb", [NQ, 128], F32)
    um_sb = nc.alloc_sbuf_tensor("um_sb", [1, NQ], F32)
    mm_sb = nc.alloc_sbuf_tensor("mm_sb", [1, 1], F32)
    out_sb = nc.alloc_sbuf_tensor("out_sb", [1, 2], F32)
    ccw_sb = nc.alloc_sbuf_tensor("ccw_sb", [128, 1], F32)

    # PSUM: bank-aligned layout (8 banks x 512 f32)
    pps = nc.alloc_psum_tensor("pps", [128, 2560], F32)       # banks 0-4
    t2ps = nc.alloc_psum_tensor("t2ps", [128, 512], F32)      # bank 5
    t2ps2 = nc.alloc_psum_tensor("t2ps2", [128, 512], F32)    # bank 6
    miscps = nc.alloc_psum_tensor("miscps", [128, 512], F32)  # bank 7

    from contextlib import ExitStack

    with ExitStack() as stack:
        block = stack.enter_context(nc.Block())
        sem_names = [
            "s_din", "s_din1", "s_tin", "s_tin1", "s_tin2", "s_tin3",
            "s_cast", "s_tcast", "s_sq", "s_isq", "s_t2mm",
            "s_t2dr", "s_t2dma", "s_const", "s_B", "s_hx", "s_hdma", "s_cwmm",
            "s_cwdr", "s_pe", "s_drain", "s_skew", "s_smm", "s_loss", "s_ar",
            "s_cc", "s_arb", "s_post", "s_rs", "s_out", "s_v", "s_ccw", "s_ccw2",
        ]
        sems = {n: stack.enter_context(nc.semaphore(n)) for n in sem_names}
        (
            s_din, s_din1, s_tin, s_tin1, s_tin2, s_tin3,
            s_cast, s_tcast, s_sq, s_isq, s_t2mm, s_t2dr,
            s_t2dma, s_const, s_B, s_hx, s_hdma, s_cwmm, s_cwdr, s_pe,
            s_drain, s_skew, s_smm, s_loss, s_ar, s_cc, s_arb, s_post,
            s_rs, s_out, s_v, s_ccw, s_ccw2,
        ) = (sems[n] for n in sem_names)

        @block.sync
        def _(sync: bass.BassEngine):
            # input loads split across the two HWDGE rings (sync + scalar);
            # this ring carries tgt c0, inp c0, tgt c2, inp c1.
            sync.dma_start(
                out=tgt32[:, 0:TCW], in_=tgt_ext[:, 0:TCW]
            ).then_inc(s_tin, 16)
            sync.dma_start(
                out=inp32[:, 0:ICW], in_=inp_ext[:, 0:ICW]
            ).then_inc(s_din, 16)
            sync.wait_ge(s_tin, 16)
            sync.dma_start(
                out=tgt32[:, 2 * TCW : 3 * TCW], in_=tgt_ext[:, 2 * TCW : 3 * TCW]
            ).then_inc(s_tin2, 16)
            sync.dma_start(
                out=inp32[:, ICW:LIN], in_=inp_ext[:, ICW:LIN]
            ).then_inc(s_din1, 16)

            # t2: SBUF flat -> DRAM -> [80,128] blocks
            sync.wait_ge(s_t2dr, NT2)
            sync.dma_start(out=t2_dram[:], in_=t2flat[:, :]).then_inc(s_t2dma, 16)
            sync.wait_ge(s_t2dma, 16)
            sync.dma_start(
                out=t2sb[0:NBLK, :],
                in_=t2_dram.ap().rearrange("(p f) -> p f", p=NBLK),
            ).then_inc(s_t2dma, 16)

            # Hx[64:81] -> htop[0:17]
            sync.wait_ge(s_hx, 1)
            sync.dma_start(out=htop[0:NQ, :], in_=hx[64 : 64 + NQ, :]).then_inc(
                s_hdma, 16
            )

            # P -> DRAM (dense), DRAM -> skewsb (stride-2177 diagonal read)
            sync.wait_ge(s_drain, 2)
            sync.dma_start(
                out=AP(pskew_dram, 0, [[PW, 128], [1, PW]]),
                in_=psb[:, :],
            ).then_inc(s_skew, 16)
            sync.wait_ge(s_skew, 16)
            sync.dma_start(
                out=skewsb[:, :],
                in_=AP(pskew_dram, 0, [[SKST, 128], [1, S]]),
            ).then_inc(s_skew, 16)

            # loss partial -> DRAM for AR
            sync.wait_ge(s_loss, 1)
            sync.dma_start(out=lossp_dram[:, :], in_=loss_sb[:, :]).then_inc(s_ar, 16)

            # AR result -> SBUF
            sync.wait_ge(s_cc, 1)
            sync.dma_start(out=lossr_sb[:, :], in_=lossr_dram[:, :]).then_inc(
                s_arb, 16
            )

            # row stats transpose: [17,2] -> [1,34] in one SBUF->SBUF DMA
            sync.wait_ge(s_post, 1)
            sync.dma_start(out=rst_sb[:, :], in_=rs_sb[:, :]).then_inc(s_rs, 16)

            # final output
            sync.wait_ge(s_out, 1)
            sync.dma_start(out=out_ext[:, :], in_=out_sb[:, :]).then_inc(s_out, 16)

        @block.gpsimd
        def _(gpsimd: bass.BassGpSimd):
            # warmup collective: absorbs ncfw/collective-stream init cost
            # while DMAs and compute run; nobody waits on its result.
            gpsimd.memset(ccw_sb[:, :], 0.0).then_inc(s_ccw2, 1)
            gpsimd.wait_ge(s_ccw2, 1)
            gpsimd.dma_start(out=ccw_in_dram[:], in_=ccw_sb[:, :]).then_inc(s_ccw, 16)
            gpsimd.wait_ge(s_ccw, 16)
            gpsimd.collective_compute(
                "AllReduce",
                mybir.AluOpType.add,
                replica_groups=[list(range(NCORES))],
                ins=[ccw_in_dram.ap().opt()],
                outs=[ccw_out_dram.ap().opt()],
            ).then_inc(s_ccw2, 1)

            # constants; sem edges between dependent ops (Q7 cores overlap)
            gpsimd.memset(maskb[:, :], 1.0).then_inc(s_const, 1)
            gpsimd.wait_ge(s_const, 1)
            # keep where (k - q) >= 0
            gpsimd.affine_select(
                out=maskb[:, :], in_=maskb[:, :],
                pattern=[[-1, NQ]], compare_op=mybir.AluOpType.is_ge,
                fill=0.0, base=0, channel_multiplier=1,
            ).then_inc(s_const, 1)
            gpsimd.wait_ge(s_const, 2)
            # keep where (63 + q - k) >= 0
            gpsimd.affine_select(
                out=maskb[:, :], in_=maskb[:, :],
                pattern=[[1, NQ]], compare_op=mybir.AluOpType.is_ge,
                fill=0.0, base=63, channel_multiplier=-1,
            ).then_inc(s_const, 1)
            gpsimd.memset(ones17[:, :], 1.0).then_inc(s_const, 1)
            gpsimd.memset(onecol[:, :], 1.0).then_inc(s_const, 1)
            gpsimd.memset(onehot[:, :], 0.0).then_inc(s_const, 1)
            gpsimd.wait_ge(s_const, 6)
            gpsimd.memset(onehot[:, NQ - 1 : NQ], 1.0).then_inc(s_const, 1)
            gpsimd.iota(
                iota_i[0:NQ, :], pattern=[[1, 128]], base=0, channel_multiplier=128
            ).then_inc(s_const, 1)
            gpsimd.memset(maskadd[:, :], 0.0).then_inc(s_const, 1)
            gpsimd.wait_ge(s_const, 9)
            # keep 0 where (2048 - 128q - r) >= 0, else big
            gpsimd.affine_select(
                out=maskadd[:, :], in_=maskadd[:, :],
                pattern=[[-1, 128]], compare_op=mybir.AluOpType.is_ge,
                fill=1.0e30, base=S - 1, channel_multiplier=-128,
            ).then_inc(s_const, 1)

            # the collective
            gpsimd.wait_ge(s_ar, 16)
            gpsimd.collective_compute(
                "AllReduce",
                mybir.AluOpType.add,
                replica_groups=[list(range(NCORES))],
                ins=[lossp_dram.ap().opt()],
                outs=[lossr_dram.ap().opt()],
            ).then_inc(s_cc, 1)

        @block.vector
        def _(vector: bass.BassEngine):
            # every DVE op in the sequential sections incs s_v and waits on
            # the previous count, giving the race detector explicit edges.
            vcnt = [0]

            def vstep(mk, *waits):
                for sem, val in waits:
                    vector.wait_ge(sem, val)
                if vcnt[0] > 0:
                    vector.wait_ge(s_v, vcnt[0])
                mk().then_inc(s_v, 1)
                vcnt[0] += 1

            # chunked casts; input scaled by -2 during cast.  First chunks
            # of both tensors first so the PE can start early.
            vstep(lambda: vector.tensor_scalar(
                out=inpbf[:, 0:ICW], in0=inp32[:, 0:ICW], scalar1=-2.0,
                scalar2=None, op0=mybir.AluOpType.mult,
            ), (s_din, 16))
            vector.wait_ge(s_v, vcnt[0])
            vector.engine_nop().then_inc(s_cast, 1)
            vstep(lambda: vector.tensor_copy(tgtbf[:, 0:TCW], tgt32[:, 0:TCW]),
                  (s_tin, 16))
            vector.wait_ge(s_v, vcnt[0])
            vector.engine_nop().then_inc(s_tcast, 1)
            vstep(lambda: vector.tensor_scalar(
                out=inpbf[:, ICW:LIN], in0=inp32[:, ICW:LIN], scalar1=-2.0,
                scalar2=None, op0=mybir.AluOpType.mult,
            ), (s_din1, 16))
            vector.wait_ge(s_v, vcnt[0])
            vector.engine_nop().then_inc(s_cast, 1)
            tsems = [s_tin, s_tin1, s_tin2, s_tin3]
            for c in range(1, TCH):
                vstep(lambda c=c: vector.tensor_copy(
                    tgtbf[:, c * TCW : (c + 1) * TCW],
                    tgt32[:, c * TCW : (c + 1) * TCW],
                ), (tsems[c], 16))
                vector.wait_ge(s_v, vcnt[0])
                vector.engine_nop().then_inc(s_tcast, 1)

            # iota cast (after gpsimd consts)
            vstep(lambda: vector.tensor_copy(iota_f[:, :], iota_i[:, :]),
                  (s_const, 10))

            # colsum1 = per-bc sum of inp^2 (from ACT chunk accums)
            vstep(lambda: vector.reduce_sum(
                colsum1[:, :], sqacc[:, 0:4], axis=mybir.AxisListType.X),
                (s_isq, 4))
            vstep(lambda: vector.tensor_copy(colsum_bf[:, :], colsum1[:, :]))
            vector.wait_ge(s_v, vcnt[0])
            vector.engine_nop().then_inc(s_B, 1)

            # winsum path: inclusive prefix over 128-wide blocks
            vector.wait_ge(s_t2dma, 32)
            src, dst = t2sb, t2sb2
            for sh in (1, 2, 4, 8, 16, 32, 64):
                vstep(lambda src=src, dst=dst, sh=sh: vector.tensor_copy(
                    dst[0:NBLK, 0:sh], src[0:NBLK, 0:sh]))
                vstep(lambda src=src, dst=dst, sh=sh: vector.tensor_tensor(
                    out=dst[0:NBLK, sh:128],
                    in0=src[0:NBLK, sh:128],
                    in1=src[0:NBLK, 0 : 128 - sh],
                    op=mybir.AluOpType.add,
                ))
                src, dst = dst, src
            incl = src  # inclusive prefix lands here after 7 swaps
            # exclusive prefix hx (rows 64..80 zeroed first: row 80 stays 0
            # for the masked q=16 tail)
            vstep(lambda: vector.memset(hx[64 : NBLK + 1, :], 0.0))
            vstep(lambda: vector.memset(hx[0:NBLK, 0:1], 0.0))
            vstep(lambda: vector.tensor_copy(hx[0:NBLK, 1:128], incl[0:NBLK, 0:127]))
            vstep(lambda: vector.tensor_copy(b_bf[0:NBLK, :], incl[0:NBLK, 127:128]))
            vector.wait_ge(s_v, vcnt[0])
            vector.engine_nop().then_inc(s_hx, 1)  # hx ready (sync DMAs htop)
            vector.engine_nop().then_inc(s_B, 1)   # B(bf16) ready for CW mm

            # loss_sb = htop - hx[0:17]
            vstep(lambda: vector.tensor_tensor(
                out=loss_sb[:, :], in0=htop[:, :], in1=hx[0:NQ, :],
                op=mybir.AluOpType.subtract,
            ), (s_hdma, 16))

            # P drain part 1 (banks 0-1) -- must precede the skew-sum wait
            vstep(lambda: vector.tensor_copy(psb[:, 0:1024], pps[:, 0:1024]),
                  (s_pe, 1))
            vector.wait_ge(s_v, vcnt[0])
            vector.engine_nop().then_inc(s_drain, 1)

            # += CW (per-partition scalar)
            vstep(lambda: vector.tensor_scalar(
                out=loss_sb[:, :], in0=loss_sb[:, :], scalar1=cw_sb[0:NQ, 0:1],
                scalar2=None, op0=mybir.AluOpType.add,
            ), (s_cwdr, 1))
            # += (-2 corr) from skew-sum psum
            vstep(lambda: vector.tensor_tensor(
                out=loss_sb[:, :], in0=loss_sb[:, :],
                in1=miscps[0:NQ, 128:256],
                op=mybir.AluOpType.add,
            ), (s_smm, 1))
            # += mask (pre-AR; 8x-summed mask still dominates)
            vstep(lambda: vector.tensor_tensor(
                out=loss_sb[:, :], in0=loss_sb[:, :], in1=maskadd[:, :],
                op=mybir.AluOpType.add,
            ), (s_const, 10))
            vector.wait_ge(s_v, vcnt[0])
            vector.engine_nop().then_inc(s_loss, 1)

            # post-AR argmin chain
            vstep(lambda: vector.tensor_reduce(
                out=rs_sb[:, 0:1], in_=lossr_sb[:, :],
                axis=mybir.AxisListType.X, op=mybir.AluOpType.min,
            ), (s_arb, 16))
            vstep(lambda: vector.tensor_scalar(
                out=u_sb[:, :], in0=lossr_sb[:, :], scalar1=rs_sb[0:NQ, 0:1],
                scalar2=65536.0, op0=mybir.AluOpType.subtract,
                op1=mybir.AluOpType.mult,
            ))
            vstep(lambda: vector.tensor_tensor(
                out=u_sb[:, :], in0=u_sb[:, :], in1=iota_f[:, :],
                op=mybir.AluOpType.add,
            ))
            vstep(lambda: vector.tensor_reduce(
                out=rs_sb[:, 1:2], in_=u_sb[:, :],
                axis=mybir.AxisListType.X, op=mybir.AluOpType.min,
            ))
            vector.wait_ge(s_v, vcnt[0])
            vector.engine_nop().then_inc(s_post, 1)

            # after transpose-DMA: global min + argmin
            rmin_v = AP(rst_sb, 0, [[2 * NQ, 1], [2, NQ]])
            ridx_v = AP(rst_sb, 1, [[2 * NQ, 1], [2, NQ]])
            vstep(lambda: vector.tensor_reduce(
                out=mm_sb[:, :], in_=rmin_v,
                axis=mybir.AxisListType.X, op=mybir.AluOpType.min,
            ), (s_rs, 16))
            vstep(lambda: vector.tensor_scalar(
                out=um_sb[:, :], in0=rmin_v, scalar1=mm_sb[0:1, 0:1],
                scalar2=65536.0, op0=mybir.AluOpType.subtract,
                op1=mybir.AluOpType.mult,
            ))
            vstep(lambda: vector.tensor_tensor(
                out=um_sb[:, :], in0=um_sb[:, :], in1=ridx_v,
                op=mybir.AluOpType.add,
            ))
            vstep(lambda: vector.tensor_reduce(
                out=out_sb[:, 1:2], in_=um_sb[:, :],
                axis=mybir.AxisListType.X, op=mybir.AluOpType.min,
            ))
            vstep(lambda: vector.tensor_scalar(
                out=out_sb[:, 0:1], in0=mm_sb[:, :], scalar1=1.0 / NTOT,
                scalar2=None, op0=mybir.AluOpType.mult,
            ))
            vector.wait_ge(s_v, vcnt[0])
            vector.engine_nop().then_inc(s_out, 1)

        @block.scalar
        def _(scalar: bass.BassEngine):
            # issue the second half of the input loads from the ACT HWDGE
            # ring so the two physical rings split the load
            tsems = [s_tin, s_tin1, s_tin2, s_tin3]
            scalar.dma_start(
                out=tgt32[:, TCW : 2 * TCW], in_=tgt_ext[:, TCW : 2 * TCW]
            ).then_inc(s_tin1, 16)
            scalar.dma_start(
                out=tgt32[:, 3 * TCW : 4 * TCW], in_=tgt_ext[:, 3 * TCW : 4 * TCW]
            ).then_inc(s_tin3, 16)

            # target squared chunks as they arrive (t2 matmuls consume these)
            for c in range(TCH):
                scalar.wait_ge(tsems[c], 16)
                if c > 0:
                    scalar.wait_ge(s_sq, c)
                scalar.activation(
                    out=sqbf[:, c * TCW : (c + 1) * TCW],
                    in_=tgt32[:, c * TCW : (c + 1) * TCW],
                    func=mybir.ActivationFunctionType.Square,
                ).then_inc(s_sq, 1)

            # inp^2 row sums in 4 chunks (square + accumulate)
            scalar.wait_ge(s_din, 16)
            for i in range(4):
                if i == 2:
                    scalar.wait_ge(s_din1, 16)
                if i > 0:
                    scalar.wait_ge(s_isq, i)
                scalar.activation(
                    out=sqscr[:, :],
                    in_=inp32[:, i * 2048 : (i + 1) * 2048],
                    func=mybir.ActivationFunctionType.Square,
                    accum_out=sqacc[:, i : i + 1],
                ).then_inc(s_isq, 1)

            # drain t2 slabs PSUM -> t2flat (t2 matmuls run late, a>=40)
            for k in range(NT2):
                scalar.wait_ge(s_t2mm, k + 1)
                bank = (t2ps, t2ps2, miscps)[k % 3]
                scalar.copy(
                    t2flat[0:1, 512 * k : 512 * (k + 1)], bank[0:1, :]
                ).then_inc(s_t2dr, 1)

            # drain CW psum -> cw_sb
            scalar.wait_ge(s_cwmm, 1)
            scalar.copy(cw_sb[0:NQ, 0:1], miscps[0:NQ, 0:1]).then_inc(s_cwdr, 1)

            # P drain part 2 (banks 2-4)
            scalar.wait_ge(s_pe, 1)
            scalar.copy(psb[:, 1024:PW], pps[:, 1024:PW]).then_inc(s_drain, 1)

        @block.tensor
        def _(tensor: bass.BassEngine):
            t2k = 0
            icast = 1   # inpbf chunks available so far
            tcast = 1   # tgtbf chunks available so far
            tensor.wait_ge(s_const, 10)
            tensor.wait_ge(s_cast, 1)
            tensor.wait_ge(s_tcast, 1)
            for a in range(AW):
                if 128 * a + 128 > icast * ICW:
                    icast += 1
                    tensor.wait_ge(s_cast, icast)
                while 128 * a + PW > tcast * TCW:
                    tcast += 1
                    tensor.wait_ge(s_tcast, tcast)
                lhsT = inpbf[:, 128 * a : 128 * (a + 1)]
                base = 128 * a
                for j in range(4):
                    tensor.matmul(
                        out=pps[:, 512 * j : 512 * (j + 1)],
                        lhsT=lhsT,
                        rhs=tgtbf[:, base + 512 * j : base + 512 * (j + 1)],
                        start=(a == 0),
                        stop=(a == AW - 1),
                    )
                mm = tensor.matmul(
                    out=pps[:, 2048 : 2048 + 128],
                    lhsT=lhsT,
                    rhs=tgtbf[:, base + 2048 : base + PW],
                    start=(a == 0),
                    stop=(a == AW - 1),
                )
                if a == AW - 1:
                    mm.then_inc(s_pe, 1)
                # interleave one t2 matmul per iteration
                if T2MM_AT <= a < T2MM_AT + NT2:
                    tensor.wait_ge(s_sq, t2k // 5 + 1)
                    if t2k >= 3:
                        tensor.wait_ge(s_t2dr, t2k - 2)
                    bank = (t2ps, t2ps2, miscps)[t2k % 3]
                    tensor.matmul(
                        out=bank[0:1, :],
                        lhsT=onecol[:, :],
                        rhs=sqbf[:, 512 * t2k : 512 * (t2k + 1)],
                        start=True,
                        stop=True,
                    ).then_inc(s_t2mm, 1)
                    t2k += 1

            # CW matmuls: cw[q] = sum_{j=q..q+63} B[j] + inp_sq
            tensor.wait_ge(s_t2dr, NT2)
            tensor.wait_ge(s_B, 2)
            tensor.matmul(
                out=miscps[0:NQ, 0:1],
                lhsT=maskb[0:NBLK, :],
                rhs=b_bf[0:NBLK, :],
                start=True, stop=False,
            )
            tensor.matmul(
                out=miscps[0:NQ, 0:1],
                lhsT=ones17[:, :],
                rhs=colsum_bf[:, :],
                start=False, stop=True,
            ).then_inc(s_cwmm, 1)

            # skew-sum matmuls: misc[q,128+r] = sum_m skew[m, 128q+r]
            tensor.wait_ge(s_skew, 32)
            tensor.wait_ge(s_cwdr, 1)
            for q in range(NQ):
                ncols = 128 if q < NQ - 1 else 1
                mm = tensor.matmul(
                    out=miscps[0:NQ, 128 : 128 + ncols],
                    lhsT=onehot[:, NQ - 1 - q : 2 * NQ - 1 - q],
                    rhs=skewsb[:, 128 * q : 128 * q + ncols],
                    start=(q == 0),
                    stop=(q == NQ - 1),
                )
                if q == NQ - 1:
                    mm.then_inc(s_smm, 1)

    return nc


_NC_CACHE = None


def _get_nc():
    global _NC_CACHE
    if _NC_CACHE is None:
        _NC_CACHE = build_bass()
    return _NC_CACHE


def make_in_maps(input, target):
    inp = np.ascontiguousarray(np.asarray(input, dtype=np.float32))
    tgt = np.ascontiguousarray(np.asarray(target, dtype=np.float32))
    per = B // NCORES
    in_maps = []
    for c in range(NCORES):
        in_maps.append(
            {
                "input": np.ascontiguousarray(
                    inp[c * per : (c + 1) * per].reshape(BC, LIN)
                ),
                "target": np.ascontiguousarray(
                    tgt[c * per : (c + 1) * per].reshape(BC, LTGT)
                ),
            }
        )
    return in_maps


LAST_RESULTS = None


def kernel(input, target, trace=False, **trace_kwargs):
    global LAST_RESULTS
    from concourse.bass_utils import run_bass_kernel_spmd

    nc = _get_nc()
    in_maps = make_in_maps(input, target)
    res = run_bass_kernel_spmd(
        nc, in_maps, core_ids=list(range(NCORES)), trace=trace, **trace_kwargs
    )
    LAST_RESULTS = res
    out = res.results[0]["out"]
    min_loss = np.float32(out[0, 0])
    min_index = np.int32(np.rint(out[0, 1]))
    return (min_loss, min_index)


if __name__ == "__main__":
    nc = build_bass()
    print("bass graph built OK")


# revision 33
# speedup vs baseline: 1.2825x; 1.2825x over previous
"""Adaptive MSE loss (min over shifts) on 8 TRN2 NeuronCores.

Full inputs: input [16,64,8192] f32, target [16,64,10240] f32.
Data-parallel over batch B: 2 batches/core -> bc=128 rows on partitions.

Per core:
  P[m,u]    = sum_a sum_bc (-2*inp_bf)[bc,128a+m] * tgt_bf[bc,128a+u]   (PE, bf16)
  -2corr[s] = sum_m P[m, m+s]          (dense DRAM write + stride-2177 read skew,
                                        then one-hot fp32 matmuls -> [17,128])
  winsum[s] = CW[q] - Hx[q,r] + Hx[q+64,r]   (s=128q+r; block prefix sums of t2)
  t2[u]     = sum_bc tgt[bc,u]^2       (ones-matmul, bf16)
  inp_sq    = sum inp^2                (ACT square+accum, folded into CW matmul)
  loss*n    = inp_sq + winsum - 2corr  -> AllReduce [17,128] -> argmin on-device.

Inputs are loaded in chunks with casts/squares pipelined per chunk so the
PE starts ~15us in; a tiny warmup collective at t=0 absorbs ncfw init.
"""

import sys
import numpy as np

sys.path.insert(0, "/opt/trn_rl_repo")

from concourse import bass, mybir  # noqa: E402
from concourse.ap import AP  # noqa: E402

F32 = mybir.dt.float32
BF16 = mybir.dt.bfloat16
I32 = mybir.dt.int32

B, C, LIN, LTGT = 16, 64, 8192, 10240
NCORES = 8
BC = (B // NCORES) * C            # 128 rows per core
S = LTGT - LIN + 1                # 2049 shifts
AW = LIN // 128                   # 64 contraction chunks
PW = 2176                         # P width (17*128)
NQ = 17                           # shift blocks (s = 128q+r)
NBLK = LTGT // 128                # 80 t2 blocks
NTOT = float(B * C * LIN)         # 8388608
SKST = PW + 1                     # 2177: skew read row stride (write is dense)

ICH = 2                           # input DMA chunks (4096 cols each)
TCH = 4                           # target DMA chunks (2560 cols each)
ICW = LIN // ICH
TCW = LTGT // TCH
T2MM_AT = 40                      # corr-loop index where t2 matmuls interleave
NT2 = LTGT // 512                 # 20 t2 slabs


def build_bass():
    nc = bass.Bass(num_devices=NCORES)

    inp_ext = nc.declare_dram_parameter("input", [BC, LIN], F32, isOutput=False)
    tgt_ext = nc.declare_dram_parameter("target", [BC, LTGT], F32, isOutput=False)
    out_ext = nc.declare_dram_parameter("out", [1, 2], F32, isOutput=True)

    t2_dram = nc.dram_tensor("t2_dram", [NBLK * 128], F32)
    pskew_dram = nc.dram_tensor("pskew_dram", [128 * PW], F32)
    lossp_dram = nc.dram_tensor("lossp_dram", [NQ, 128], F32)
    lossr_dram = nc.dram_tensor("lossr_dram", [NQ, 128], F32, addr_space="Shared")
    ccw_in_dram = nc.dram_tensor("ccw_in_dram", [128], F32)
    ccw_out_dram = nc.dram_tensor("ccw_out_dram", [128], F32, addr_space="Shared")

    # SBUF
    inp32 = nc.alloc_sbuf_tensor("inp32", [BC, LIN], F32)
    tgt32 = nc.alloc_sbuf_tensor("tgt32", [BC, LTGT], F32)
    inpbf = nc.alloc_sbuf_tensor("inpbf", [BC, LIN], BF16)
    tgtbf = nc.alloc_sbuf_tensor("tgtbf", [BC, LTGT], BF16)
    sqbf = nc.alloc_sbuf_tensor("sqbf", [BC, LTGT], BF16)
    sqscr = nc.alloc_sbuf_tensor("sqscr", [BC, 2048], BF16)
    sqacc = nc.alloc_sbuf_tensor("sqacc", [BC, 8], F32)
    colsum1 = nc.alloc_sbuf_tensor("colsum1", [BC, 1], F32)
    t2flat = nc.alloc_sbuf_tensor("t2flat", [1, LTGT], F32)
    psb_off = nc.sbuf_base
    psb = nc.alloc_sbuf_tensor("psb", [128, PW], F32)
    # skewsb aliases psb: psb is dead once its DMA-out completes, and the
    # skew-read DMA that fills skewsb is sem-ordered after that DMA-out.
    skewsb = nc.alloc_sbuf_tensor_at("skewsb", [128, S], F32, offset=psb_off)
    t2sb = nc.alloc_sbuf_tensor("t2sb", [128, 128], F32)
    t2sb2 = nc.alloc_sbuf_tensor("t2sb2", [128, 128], F32)
    hx = nc.alloc_sbuf_tensor("hx", [128, 128], F32)
    htop = nc.alloc_sbuf_tensor("htop", [NQ, 128], F32)
    loss_sb = nc.alloc_sbuf_tensor("loss_sb", [NQ, 128], F32)
    lossr_sb = nc.alloc_sbuf_tensor("lossr_sb", [NQ, 128], F32)
    cw_sb = nc.alloc_sbuf_tensor("cw_sb", [NQ, 1], F32)
    b_bf = nc.alloc_sbuf_tensor("b_bf", [128, 1], BF16)
    colsum_bf = nc.alloc_sbuf_tensor("colsum_bf", [128, 1], BF16)
    iota_i = nc.alloc_sbuf_tensor("iota_i", [NQ, 128], I32)
    iota_f = nc.alloc_sbuf_tensor("iota_f", [NQ, 128], F32)
    maskb = nc.alloc_sbuf_tensor("maskb", [128, NQ], BF16)
    ones17 = nc.alloc_sbuf_tensor("ones17", [128, NQ], BF16)
    onecol = nc.alloc_sbuf_tensor("onecol", [128, 1], BF16)
    onehot = nc.alloc_sbuf_tensor("onehot", [128, 2 * NQ - 1], F32)
    maskadd = nc.alloc_sbuf_tensor("maskadd", [NQ, 128], F32)
    rs_sb = nc.alloc_sbuf_tensor("rs_sb", [NQ, 2], F32)
    rst_sb = nc.alloc_sbuf_tensor("rst_sb", [1, 2 * NQ], F32)
    u_sb = nc.alloc_sbuf_tensor("u_sb", [NQ, 128], F32)
    um_sb = nc.alloc_sbuf_tensor("um_sb", [1, NQ], F32)
    mm_sb = nc.alloc_sbuf_tensor("mm_sb", [1, 1], F32)
    out_sb = nc.alloc_sbuf_tensor("out_sb", [1, 2], F32)
    ccw_sb = nc.alloc_sbuf_tensor("ccw_sb", [128, 1], F32)

    # PSUM: bank-aligned layout (8 banks x 512 f32)
    pps = nc.alloc_psum_tensor("pps", [128, 2560], F32)       # banks 0-4
    t2ps = nc.alloc_psum_tensor("t2ps", [128, 512], F32)      # bank 5
    t2ps2 = nc.alloc_psum_tensor("t2ps2", [128, 512], F32)    # bank 6
    miscps = nc.alloc_psum_tensor("miscps", [128, 512], F32)  # bank 7

    from contextlib import ExitStack

    with ExitStack() as stack:
        block = stack.enter_context(nc.Block())
        sem_names = [
            "s_din", "s_din1", "s_tin", "s_tin1", "s_tin2", "s_tin3",
            "s_cast", "s_tcast", "s_sq", "s_isq", "s_t2mm",
            "s_t2dr", "s_t2dma", "s_const", "s_B", "s_hx", "s_hdma", "s_cwmm",
            "s_cwdr", "s_pe", "s_drain", "s_skew", "s_smm", "s_loss", "s_ar",
            "s_cc", "s_arb", "s_post", "s_rs", "s_out", "s_v", "s_ccw", "s_ccw2",
        ]
        sems = {n: stack.enter_context(nc.semaphore(n)) for n in sem_names}
        (
            s_din, s_din1, s_tin, s_tin1, s_tin2, s_tin3,
            s_cast, s_tcast, s_sq, s_isq, s_t2mm, s_t2dr,
            s_t2dma, s_const, s_B, s_hx, s_hdma, s_cwmm, s_cwdr, s_pe,
            s_drain, s_skew, s_smm, s_loss, s_ar, s_cc, s_arb, s_post,
            s_rs, s_out, s_v, s_ccw, s_ccw2,
        ) = (sems[n] for n in sem_names)

        @block.sync
        def _(sync: bass.BassEngine):
            # input loads split across the two HWDGE rings; this ring
            # carries tgt c0, inp c0, tgt c1, inp c1 (earliest-needed);
            # the ACT ring (whose boilerplate delays issue ~10us) carries
            # the late chunks tgt c2, tgt c3.
            sync.dma_start(
                out=tgt32[:, 0:TCW], in_=tgt_ext[:, 0:TCW]
            ).then_inc(s_tin, 16)
            sync.dma_start(
                out=inp32[:, 0:ICW], in_=inp_ext[:, 0:ICW]
            ).then_inc(s_din, 16)
            sync.wait_ge(s_tin, 16)
            sync.dma_start(
                out=tgt32[:, TCW : 2 * TCW], in_=tgt_ext[:, TCW : 2 * TCW]
            ).then_inc(s_tin1, 16)
            sync.dma_start(
                out=inp32[:, ICW:LIN], in_=inp_ext[:, ICW:LIN]
            ).then_inc(s_din1, 16)

            # t2: SBUF flat -> DRAM -> [80,128] blocks
            sync.wait_ge(s_t2dr, NT2)
            sync.dma_start(out=t2_dram[:], in_=t2flat[:, :]).then_inc(s_t2dma, 16)
            sync.wait_ge(s_t2dma, 16)
            sync.dma_start(
                out=t2sb[0:NBLK, :],
                in_=t2_dram.ap().rearrange("(p f) -> p f", p=NBLK),
            ).then_inc(s_t2dma, 16)

            # Hx[64:81] -> htop[0:17]
            sync.wait_ge(s_hx, 1)
            sync.dma_start(out=htop[0:NQ, :], in_=hx[64 : 64 + NQ, :]).then_inc(
                s_hdma, 16
            )

            # P -> DRAM (dense), DRAM -> skewsb (stride-2177 diagonal read)
            sync.wait_ge(s_drain, 2)
            sync.dma_start(
                out=AP(pskew_dram, 0, [[PW, 128], [1, PW]]),
                in_=psb[:, :],
            ).then_inc(s_skew, 16)
            sync.wait_ge(s_skew, 16)
            sync.dma_start(
                out=skewsb[:, :],
                in_=AP(pskew_dram, 0, [[SKST, 128], [1, S]]),
            ).then_inc(s_skew, 16)

            # loss partial -> DRAM for AR
            sync.wait_ge(s_loss, 1)
            sync.dma_start(out=lossp_dram[:, :], in_=loss_sb[:, :]).then_inc(s_ar, 16)

            # AR result -> SBUF
            sync.wait_ge(s_cc, 1)
            sync.dma_start(out=lossr_sb[:, :], in_=lossr_dram[:, :]).then_inc(
                s_arb, 16
            )

            # row stats transpose: [17,2] -> [1,34] in one SBUF->SBUF DMA
            sync.wait_ge(s_post, 1)
            sync.dma_start(out=rst_sb[:, :], in_=rs_sb[:, :]).then_inc(s_rs, 16)

            # final output
            sync.wait_ge(s_out, 1)
            sync.dma_start(out=out_ext[:, :], in_=out_sb[:, :]).then_inc(s_out, 16)

        @block.gpsimd
        def _(gpsimd: bass.BassGpSimd):
            # warmup collective: absorbs ncfw/collective-stream init cost
            # while DMAs and compute run; nobody waits on its result.
            gpsimd.memset(ccw_sb[:, :], 0.0).then_inc(s_ccw2, 1)
            gpsimd.wait_ge(s_ccw2, 1)
            gpsimd.dma_start(out=ccw_in_dram[:], in_=ccw_sb[:, :]).then_inc(s_ccw, 16)
            gpsimd.wait_ge(s_ccw, 16)
            gpsimd.collective_compute(
                "AllReduce",
                mybir.AluOpType.add,
                replica_groups=[list(range(NCORES))],
                ins=[ccw_in_dram.ap().opt()],
                outs=[ccw_out_dram.ap().opt()],
            ).then_inc(s_ccw2, 1)

            # constants; sem edges between dependent ops (Q7 cores overlap)
            gpsimd.memset(maskb[:, :], 1.0).then_inc(s_const, 1)
            gpsimd.wait_ge(s_const, 1)
            # keep where (k - q) >= 0
            gpsimd.affine_select(
                out=maskb[:, :], in_=maskb[:, :],
                pattern=[[-1, NQ]], compare_op=mybir.AluOpType.is_ge,
                fill=0.0, base=0, channel_multiplier=1,
            ).then_inc(s_const, 1)
            gpsimd.wait_ge(s_const, 2)
            # keep where (63 + q - k) >= 0
            gpsimd.affine_select(
                out=maskb[:, :], in_=maskb[:, :],
                pattern=[[1, NQ]], compare_op=mybir.AluOpType.is_ge,
                fill=0.0, base=63, channel_multiplier=-1,
            ).then_inc(s_const, 1)
            gpsimd.memset(ones17[:, :], 1.0).then_inc(s_const, 1)
            gpsimd.memset(onecol[:, :], 1.0).then_inc(s_const, 1)
            gpsimd.memset(onehot[:, :], 0.0).then_inc(s_const, 1)
            gpsimd.wait_ge(s_const, 6)
            gpsimd.memset(onehot[:, NQ - 1 : NQ], 1.0).then_inc(s_const, 1)
            gpsimd.iota(
                iota_i[0:NQ, :], pattern=[[1, 128]], base=0, channel_multiplier=128
            ).then_inc(s_const, 1)
            gpsimd.memset(maskadd[:, :], 0.0).then_inc(s_const, 1)
            gpsimd.wait_ge(s_const, 9)
            # keep 0 where (2048 - 128q - r) >= 0, else big
            gpsimd.affine_select(
                out=maskadd[:, :], in_=maskadd[:, :],
                pattern=[[-1, 128]], compare_op=mybir.AluOpType.is_ge,
                fill=1.0e30, base=S - 1, channel_multiplier=-128,
            ).then_inc(s_const, 1)

            # the collective
            gpsimd.wait_ge(s_ar, 16)
            gpsimd.collective_compute(
                "AllReduce",
                mybir.AluOpType.add,
                replica_groups=[list(range(NCORES))],
                ins=[lossp_dram.ap().opt()],
                outs=[lossr_dram.ap().opt()],
            ).then_inc(s_cc, 1)

        @block.vector
        def _(vector: bass.BassEngine):
            # every DVE op in the sequential sections incs s_v and waits on
            # the previous count, giving the race detector explicit edges.
            vcnt = [0]

            def vstep(mk, *waits):
                for sem, val in waits:
                    vector.wait_ge(sem, val)
                if vcnt[0] > 0:
                    vector.wait_ge(s_v, vcnt[0])
                mk().then_inc(s_v, 1)
                vcnt[0] += 1

            # chunked casts; input scaled by -2 during cast.  First chunks
            # of both tensors first so the PE can start early.
            vstep(lambda: vector.tensor_scalar(
                out=inpbf[:, 0:ICW], in0=inp32[:, 0:ICW], scalar1=-2.0,
                scalar2=None, op0=mybir.AluOpType.mult,
            ), (s_din, 16))
            vector.wait_ge(s_v, vcnt[0])
            vector.engine_nop().then_inc(s_cast, 1)
            vstep(lambda: vector.tensor_copy(tgtbf[:, 0:TCW], tgt32[:, 0:TCW]),
                  (s_tin, 16))
            vector.wait_ge(s_v, vcnt[0])
            vector.engine_nop().then_inc(s_tcast, 1)
            vstep(lambda: vector.tensor_scalar(
                out=inpbf[:, ICW:LIN], in0=inp32[:, ICW:LIN], scalar1=-2.0,
                scalar2=None, op0=mybir.AluOpType.mult,
            ), (s_din1, 16))
            vector.wait_ge(s_v, vcnt[0])
            vector.engine_nop().then_inc(s_cast, 1)
            tsems = [s_tin, s_tin1, s_tin2, s_tin3]
            for c in range(1, TCH):
                vstep(lambda c=c: vector.tensor_copy(
                    tgtbf[:, c * TCW : (c + 1) * TCW],
                    tgt32[:, c * TCW : (c + 1) * TCW],
                ), (tsems[c], 16))
                vector.wait_ge(s_v, vcnt[0])
                vector.engine_nop().then_inc(s_tcast, 1)

            # iota cast (after gpsimd consts)
            vstep(lambda: vector.tensor_copy(iota_f[:, :], iota_i[:, :]),
                  (s_const, 10))

            # colsum1 = per-bc sum of inp^2 (from ACT chunk accums)
            vstep(lambda: vector.reduce_sum(
                colsum1[:, :], sqacc[:, 0:4], axis=mybir.AxisListType.X),
                (s_isq, 4))
            vstep(lambda: vector.tensor_copy(colsum_bf[:, :], colsum1[:, :]))
            vector.wait_ge(s_v, vcnt[0])
            vector.engine_nop().then_inc(s_B, 1)

            # winsum path: inclusive prefix over 128-wide blocks
            vector.wait_ge(s_t2dma, 32)
            src, dst = t2sb, t2sb2
            for sh in (1, 2, 4, 8, 16, 32, 64):
                vstep(lambda src=src, dst=dst, sh=sh: vector.tensor_copy(
                    dst[0:NBLK, 0:sh], src[0:NBLK, 0:sh]))
                vstep(lambda src=src, dst=dst, sh=sh: vector.tensor_tensor(
                    out=dst[0:NBLK, sh:128],
                    in0=src[0:NBLK, sh:128],
                    in1=src[0:NBLK, 0 : 128 - sh],
                    op=mybir.AluOpType.add,
                ))
                src, dst = dst, src
            incl = src  # inclusive prefix lands here after 7 swaps
            # exclusive prefix hx (rows 64..80 zeroed first: row 80 stays 0
            # for the masked q=16 tail)
            vstep(lambda: vector.memset(hx[64 : NBLK + 1, :], 0.0))
            vstep(lambda: vector.memset(hx[0:NBLK, 0:1], 0.0))
            vstep(lambda: vector.tensor_copy(hx[0:NBLK, 1:128], incl[0:NBLK, 0:127]))
            vstep(lambda: vector.tensor_copy(b_bf[0:NBLK, :], incl[0:NBLK, 127:128]))
            vector.wait_ge(s_v, vcnt[0])
            vector.engine_nop().then_inc(s_hx, 1)  # hx ready (sync DMAs htop)
            vector.engine_nop().then_inc(s_B, 1)   # B(bf16) ready for CW mm

            # loss_sb = htop - hx[0:17]
            vstep(lambda: vector.tensor_tensor(
                out=loss_sb[:, :], in0=htop[:, :], in1=hx[0:NQ, :],
                op=mybir.AluOpType.subtract,
            ), (s_hdma, 16))

            # P drain part 1 (banks 0-1) -- must precede the skew-sum wait
            vstep(lambda: vector.tensor_copy(psb[:, 0:1024], pps[:, 0:1024]),
                  (s_pe, 1))
            vector.wait_ge(s_v, vcnt[0])
            vector.engine_nop().then_inc(s_drain, 1)

            # += CW (per-partition scalar)
            vstep(lambda: vector.tensor_scalar(
                out=loss_sb[:, :], in0=loss_sb[:, :], scalar1=cw_sb[0:NQ, 0:1],
                scalar2=None, op0=mybir.AluOpType.add,
            ), (s_cwdr, 1))
            # += (-2 corr) from skew-sum psum
            vstep(lambda: vector.tensor_tensor(
                out=loss_sb[:, :], in0=loss_sb[:, :],
                in1=miscps[0:NQ, 128:256],
                op=mybir.AluOpType.add,
            ), (s_smm, 1))
            # += mask (pre-AR; 8x-summed mask still dominates)
            vstep(lambda: vector.tensor_tensor(
                out=loss_sb[:, :], in0=loss_sb[:, :], in1=maskadd[:, :],
                op=mybir.AluOpType.add,
            ), (s_const, 10))
            vector.wait_ge(s_v, vcnt[0])
            vector.engine_nop().then_inc(s_loss, 1)

            # post-AR argmin chain
            vstep(lambda: vector.tensor_reduce(
                out=rs_sb[:, 0:1], in_=lossr_sb[:, :],
                axis=mybir.AxisListType.X, op=mybir.AluOpType.min,
            ), (s_arb, 16))
            vstep(lambda: vector.tensor_scalar(
                out=u_sb[:, :], in0=lossr_sb[:, :], scalar1=rs_sb[0:NQ, 0:1],
                scalar2=65536.0, op0=mybir.AluOpType.subtract,
                op1=mybir.AluOpType.mult,
            ))
            vstep(lambda: vector.tensor_tensor(
                out=u_sb[:, :], in0=u_sb[:, :], in1=iota_f[:, :],
                op=mybir.AluOpType.add,
            ))
            vstep(lambda: vector.tensor_reduce(
                out=rs_sb[:, 1:2], in_=u_sb[:, :],
                axis=mybir.AxisListType.X, op=mybir.AluOpType.min,
            ))
            vector.wait_ge(s_v, vcnt[0])
            vector.engine_nop().then_inc(s_post, 1)

            # after transpose-DMA: global min + argmin
            rmin_v = AP(rst_sb, 0, [[2 * NQ, 1], [2, NQ]])
            ridx_v = AP(rst_sb, 1, [[2 * NQ, 1], [2, NQ]])
            vstep(lambda: vector.tensor_reduce(
                out=mm_sb[:, :], in_=rmin_v,
                axis=mybir.AxisListType.X, op=mybir.AluOpType.min,
            ), (s_rs, 16))
            vstep(lambda: vector.tensor_scalar(
                out=um_sb[:, :], in0=rmin_v, scalar1=mm_sb[0:1, 0:1],
                scalar2=65536.0, op0=mybir.AluOpType.subtract,
                op1=mybir.AluOpType.mult,
            ))
            vstep(lambda: vector.tensor_tensor(
                out=um_sb[:, :], in0=um_sb[:, :], in1=ridx_v,
                op=mybir.AluOpType.add,
            ))
            vstep(lambda: vector.tensor_reduce(
                out=out_sb[:, 1:2], in_=um_sb[:, :],
                axis=mybir.AxisListType.X, op=mybir.AluOpType.min,
            ))
            vstep(lambda: vector.tensor_scalar(
                out=out_sb[:, 0:1], in0=mm_sb[:, :], scalar1=1.0 / NTOT,
                scalar2=None, op0=mybir.AluOpType.mult,
            ))
            vector.wait_ge(s_v, vcnt[0])
            vector.engine_nop().then_inc(s_out, 1)

        @block.scalar
        def _(scalar: bass.BassEngine):
            # issue the second half of the input loads from the ACT HWDGE
            # ring so the two physical rings split the load
            tsems = [s_tin, s_tin1, s_tin2, s_tin3]
            scalar.dma_start(
                out=tgt32[:, 2 * TCW : 3 * TCW], in_=tgt_ext[:, 2 * TCW : 3 * TCW]
            ).then_inc(s_tin2, 16)
            scalar.dma_start(
                out=tgt32[:, 3 * TCW : 4 * TCW], in_=tgt_ext[:, 3 * TCW : 4 * TCW]
            ).then_inc(s_tin3, 16)

            # target squared chunks as they arrive (t2 matmuls consume these)
            for c in range(TCH):
                scalar.wait_ge(tsems[c], 16)
                if c > 0:
                    scalar.wait_ge(s_sq, c)
                scalar.activation(
                    out=sqbf[:, c * TCW : (c + 1) * TCW],
                    in_=tgt32[:, c * TCW : (c + 1) * TCW],
                    func=mybir.ActivationFunctionType.Square,
                ).then_inc(s_sq, 1)

            # inp^2 row sums in 4 chunks (square + accumulate)
            scalar.wait_ge(s_din, 16)
            for i in range(4):
                if i == 2:
                    scalar.wait_ge(s_din1, 16)
                if i > 0:
                    scalar.wait_ge(s_isq, i)
                scalar.activation(
                    out=sqscr[:, :],
                    in_=inp32[:, i * 2048 : (i + 1) * 2048],
                    func=mybir.ActivationFunctionType.Square,
                    accum_out=sqacc[:, i : i + 1],
                ).then_inc(s_isq, 1)

            # drain t2 slabs PSUM -> t2flat (t2 matmuls run late, a>=40)
            for k in range(NT2):
                scalar.wait_ge(s_t2mm, k + 1)
                bank = (t2ps, t2ps2, miscps)[k % 3]
                scalar.copy(
                    t2flat[0:1, 512 * k : 512 * (k + 1)], bank[0:1, :]
                ).then_inc(s_t2dr, 1)

            # drain CW psum -> cw_sb
            scalar.wait_ge(s_cwmm, 1)
            scalar.copy(cw_sb[0:NQ, 0:1], miscps[0:NQ, 0:1]).then_inc(s_cwdr, 1)

            # P drain part 2 (banks 2-4)
            scalar.wait_ge(s_pe, 1)
            scalar.copy(psb[:, 1024:PW], pps[:, 1024:PW]).then_inc(s_drain, 1)

        @block.tensor
        def _(tensor: bass.BassEngine):
            t2k = 0
            icast = 1   # inpbf chunks available so far
            tcast = 1   # tgtbf chunks available so far
            tensor.wait_ge(s_const, 10)
            tensor.wait_ge(s_cast, 1)
            tensor.wait_ge(s_tcast, 1)
            for a in range(AW):
                if 128 * a + 128 > icast * ICW:
                    icast += 1
                    tensor.wait_ge(s_cast, icast)
                while 128 * a + PW > tcast * TCW:
                    tcast += 1
                    tensor.wait_ge(s_tcast, tcast)
                lhsT = inpbf[:, 128 * a : 128 * (a + 1)]
                base = 128 * a
                for j in range(4):
                    tensor.matmul(
                        out=pps[:, 512 * j : 512 * (j + 1)],
                        lhsT=lhsT,
                        rhs=tgtbf[:, base + 512 * j : base + 512 * (j + 1)],
                        start=(a == 0),
                        stop=(a == AW - 1),
                    )
                mm = tensor.matmul(
                    out=pps[:, 2048 : 2048 + 128],
                    lhsT=lhsT,
                    rhs=tgtbf[:, base + 2048 : base + PW],
                    start=(a == 0),
                    stop=(a == AW - 1),
                )
                if a == AW - 1:
                    mm.then_inc(s_pe, 1)
                # interleave one t2 matmul per iteration
                if T2MM_AT <= a < T2MM_AT + NT2:
                    tensor.wait_ge(s_sq, t2k // 5 + 1)
                    if t2k >= 3:
                        tensor.wait_ge(s_t2dr, t2k - 2)
                    bank = (t2ps, t2ps2, miscps)[t2k % 3]
                    tensor.matmul(
                        out=bank[0:1, :],
                        lhsT=onecol[:, :],
                        rhs=sqbf[:, 512 * t2k : 512 * (t2k + 1)],
                        start=True,
                        stop=True,
                    ).then_inc(s_t2mm, 1)
                    t2k += 1

            # CW matmuls: cw[q] = sum_{j=q..q+63} B[j] + inp_sq
            tensor.wait_ge(s_t2dr, NT2)
            tensor.wait_ge(s_B, 2)
            tensor.matmul(
                out=miscps[0:NQ, 0:1],
                lhsT=maskb[0:NBLK, :],
                rhs=b_bf[0:NBLK, :],
                start=True, stop=False,
            )
            tensor.matmul(
                out=miscps[0:NQ, 0:1],
                lhsT=ones17[:, :],
                rhs=colsum_bf[:, :],
                start=False, stop=True,
            ).then_inc(s_cwmm, 1)

            # skew-sum matmuls: misc[q,128+r] = sum_m skew[m, 128q+r]
            tensor.wait_ge(s_skew, 32)
            tensor.wait_ge(s_cwdr, 1)
            for q in range(NQ):
                ncols = 128 if q < NQ - 1 else 1
                mm = tensor.matmul(
                    out=miscps[0:NQ, 128 : 128 + ncols],
                    lhsT=onehot[:, NQ - 1 - q : 2 * NQ - 1 - q],
                    rhs=skewsb[:, 128 * q : 128 * q + ncols],
                    start=(q == 0),
                    stop=(q == NQ - 1),
                )
                if q == NQ - 1:
                    mm.then_inc(s_smm, 1)

    return nc


_NC_CACHE = None


def _get_nc():
    global _NC_CACHE
    if _NC_CACHE is None:
        _NC_CACHE = build_bass()
    return _NC_CACHE


def make_in_maps(input, target):
    inp = np.ascontiguousarray(np.asarray(input, dtype=np.float32))
    tgt = np.ascontiguousarray(np.asarray(target, dtype=np.float32))
    per = B // NCORES
    in_maps = []
    for c in range(NCORES):
        in_maps.append(
            {
                "input": np.ascontiguousarray(
                    inp[c * per : (c + 1) * per].reshape(BC, LIN)
                ),
                "target": np.ascontiguousarray(
                    tgt[c * per : (c + 1) * per].reshape(BC, LTGT)
                ),
            }
        )
    return in_maps


LAST_RESULTS = None


def kernel(input, target, trace=False, **trace_kwargs):
    global LAST_RESULTS
    from concourse.bass_utils import run_bass_kernel_spmd

    nc = _get_nc()
    in_maps = make_in_maps(input, target)
    res = run_bass_kernel_spmd(
        nc, in_maps, core_ids=list(range(NCORES)), trace=trace, **trace_kwargs
    )
    LAST_RESULTS = res
    out = res.results[0]["out"]
    min_loss = np.float32(out[0, 0])
    min_index = np.int32(np.rint(out[0, 1]))
    return (min_loss, min_index)


if __name__ == "__main__":
    nc = build_bass()
    print("bass graph built OK")


# revision 34
# speedup vs baseline: 1.3046x; 1.0172x over previous
"""Adaptive MSE loss (min over shifts) on 8 TRN2 NeuronCores.

Full inputs: input [16,64,8192] f32, target [16,64,10240] f32.
Data-parallel over batch B: 2 batches/core -> bc=128 rows on partitions.

Per core:
  P[m,u]    = sum_a sum_bc (-2*inp_bf)[bc,128a+m] * tgt_bf[bc,128a+u]   (PE, bf16)
  -2corr[s] = sum_m P[m, m+s]          (dense DRAM write + stride-2177 read skew,
                                        then one-hot fp32 matmuls -> [17,128])
  winsum[s] = CW[q] - Hx[q,r] + Hx[q+64,r]   (s=128q+r; block prefix sums of t2)
  t2[u]     = sum_bc tgt[bc,u]^2       (ones-matmul, bf16)
  inp_sq    = sum inp^2                (ACT square+accum, folded into CW matmul)
  loss*n    = inp_sq + winsum - 2corr  -> AllReduce [17,128] -> argmin on-device.

Inputs are loaded in chunks with casts/squares pipelined per chunk so the
PE starts ~15us in; a tiny warmup collective at t=0 absorbs ncfw init.
"""

import sys
import numpy as np

sys.path.insert(0, "/opt/trn_rl_repo")

from concourse import bass, mybir  # noqa: E402
from concourse.ap import AP  # noqa: E402

F32 = mybir.dt.float32
BF16 = mybir.dt.bfloat16
I32 = mybir.dt.int32

B, C, LIN, LTGT = 16, 64, 8192, 10240
NCORES = 8
BC = (B // NCORES) * C            # 128 rows per core
S = LTGT - LIN + 1                # 2049 shifts
AW = LIN // 128                   # 64 contraction chunks
PW = 2176                         # P width (17*128)
NQ = 17                           # shift blocks (s = 128q+r)
NBLK = LTGT // 128                # 80 t2 blocks
NTOT = float(B * C * LIN)         # 8388608
SKST = PW + 1                     # 2177: skew read row stride (write is dense)

ICH = 2                           # input DMA chunks (4096 cols each)
TCH = 4                           # target DMA chunks (2560 cols each)
ICW = LIN // ICH
TCW = LTGT // TCH
T2MM_AT = 40                      # corr-loop index where t2 matmuls interleave
NT2 = LTGT // 512                 # 20 t2 slabs


def build_bass():
    nc = bass.Bass(num_devices=NCORES)

    inp_ext = nc.declare_dram_parameter("input", [BC, LIN], F32, isOutput=False)
    tgt_ext = nc.declare_dram_parameter("target", [BC, LTGT], F32, isOutput=False)
    out_ext = nc.declare_dram_parameter("out", [1, 2], F32, isOutput=True)

    t2_dram = nc.dram_tensor("t2_dram", [NBLK * 128], F32)
    pskew_dram = nc.dram_tensor("pskew_dram", [128 * PW], BF16)
    lossp_dram = nc.dram_tensor("lossp_dram", [NQ, 128], F32)
    lossr_dram = nc.dram_tensor("lossr_dram", [NQ, 128], F32, addr_space="Shared")
    ccw_in_dram = nc.dram_tensor("ccw_in_dram", [128], F32)
    ccw_out_dram = nc.dram_tensor("ccw_out_dram", [128], F32, addr_space="Shared")

    # SBUF
    inp32 = nc.alloc_sbuf_tensor("inp32", [BC, LIN], F32)
    tgt32 = nc.alloc_sbuf_tensor("tgt32", [BC, LTGT], F32)
    inpbf = nc.alloc_sbuf_tensor("inpbf", [BC, LIN], BF16)
    tgtbf = nc.alloc_sbuf_tensor("tgtbf", [BC, LTGT], BF16)
    sqbf = nc.alloc_sbuf_tensor("sqbf", [BC, LTGT], BF16)
    sqscr = nc.alloc_sbuf_tensor("sqscr", [BC, 2048], BF16)
    sqacc = nc.alloc_sbuf_tensor("sqacc", [BC, 8], F32)
    colsum1 = nc.alloc_sbuf_tensor("colsum1", [BC, 1], F32)
    t2flat = nc.alloc_sbuf_tensor("t2flat", [1, LTGT], F32)
    psb_off = nc.sbuf_base
    psb = nc.alloc_sbuf_tensor("psb", [128, PW], BF16)
    # skewsb aliases psb: psb is dead once its DMA-out completes, and the
    # skew-read DMA that fills skewsb is sem-ordered after that DMA-out.
    skewsb = nc.alloc_sbuf_tensor_at("skewsb", [128, S], BF16, offset=psb_off)
    t2sb = nc.alloc_sbuf_tensor("t2sb", [128, 128], F32)
    t2sb2 = nc.alloc_sbuf_tensor("t2sb2", [128, 128], F32)
    hx = nc.alloc_sbuf_tensor("hx", [128, 128], F32)
    htop = nc.alloc_sbuf_tensor("htop", [NQ, 128], F32)
    loss_sb = nc.alloc_sbuf_tensor("loss_sb", [NQ, 128], F32)
    lossr_sb = nc.alloc_sbuf_tensor("lossr_sb", [NQ, 128], F32)
    cw_sb = nc.alloc_sbuf_tensor("cw_sb", [NQ, 1], F32)
    b_bf = nc.alloc_sbuf_tensor("b_bf", [128, 1], BF16)
    colsum_bf = nc.alloc_sbuf_tensor("colsum_bf", [128, 1], BF16)
    iota_i = nc.alloc_sbuf_tensor("iota_i", [NQ, 128], I32)
    iota_f = nc.alloc_sbuf_tensor("iota_f", [NQ, 128], F32)
    maskb = nc.alloc_sbuf_tensor("maskb", [128, NQ], BF16)
    ones17 = nc.alloc_sbuf_tensor("ones17", [128, NQ], BF16)
    onecol = nc.alloc_sbuf_tensor("onecol", [128, 1], BF16)
    onehot = nc.alloc_sbuf_tensor("onehot", [128, 2 * NQ - 1], BF16)
    maskadd = nc.alloc_sbuf_tensor("maskadd", [NQ, 128], F32)
    rs_sb = nc.alloc_sbuf_tensor("rs_sb", [NQ, 2], F32)
    rst_sb = nc.alloc_sbuf_tensor("rst_sb", [1, 2 * NQ], F32)
    u_sb = nc.alloc_sbuf_tensor("u_sb", [NQ, 128], F32)
    um_sb = nc.alloc_sbuf_tensor("um_sb", [1, NQ], F32)
    mm_sb = nc.alloc_sbuf_tensor("mm_sb", [1, 1], F32)
    out_sb = nc.alloc_sbuf_tensor("out_sb", [1, 2], F32)
    ccw_sb = nc.alloc_sbuf_tensor("ccw_sb", [128, 1], F32)

    # PSUM: bank-aligned layout (8 banks x 512 f32)
    pps = nc.alloc_psum_tensor("pps", [128, 2560], F32)       # banks 0-4
    t2ps = nc.alloc_psum_tensor("t2ps", [128, 512], F32)      # bank 5
    t2ps2 = nc.alloc_psum_tensor("t2ps2", [128, 512], F32)    # bank 6
    miscps = nc.alloc_psum_tensor("miscps", [128, 512], F32)  # bank 7

    from contextlib import ExitStack

    with ExitStack() as stack:
        block = stack.enter_context(nc.Block())
        sem_names = [
            "s_din", "s_din1", "s_tin", "s_tin1", "s_tin2", "s_tin3",
            "s_cast", "s_tcast", "s_sq", "s_isq", "s_t2mm",
            "s_t2dr", "s_t2dma", "s_const", "s_B", "s_hx", "s_hdma", "s_cwmm",
            "s_cwdr", "s_pe", "s_drain", "s_skew", "s_smm", "s_loss", "s_ar",
            "s_cc", "s_arb", "s_post", "s_rs", "s_out", "s_v", "s_ccw", "s_ccw2",
        ]
        sems = {n: stack.enter_context(nc.semaphore(n)) for n in sem_names}
        (
            s_din, s_din1, s_tin, s_tin1, s_tin2, s_tin3,
            s_cast, s_tcast, s_sq, s_isq, s_t2mm, s_t2dr,
            s_t2dma, s_const, s_B, s_hx, s_hdma, s_cwmm, s_cwdr, s_pe,
            s_drain, s_skew, s_smm, s_loss, s_ar, s_cc, s_arb, s_post,
            s_rs, s_out, s_v, s_ccw, s_ccw2,
        ) = (sems[n] for n in sem_names)

        @block.sync
        def _(sync: bass.BassEngine):
            # input loads split across the two HWDGE rings; this ring
            # carries tgt c0, inp c0, tgt c1, inp c1 (earliest-needed);
            # the ACT ring (whose boilerplate delays issue ~10us) carries
            # the late chunks tgt c2, tgt c3.
            sync.dma_start(
                out=tgt32[:, 0:TCW], in_=tgt_ext[:, 0:TCW]
            ).then_inc(s_tin, 16)
            sync.dma_start(
                out=inp32[:, 0:ICW], in_=inp_ext[:, 0:ICW]
            ).then_inc(s_din, 16)
            sync.wait_ge(s_tin, 16)
            sync.dma_start(
                out=tgt32[:, TCW : 2 * TCW], in_=tgt_ext[:, TCW : 2 * TCW]
            ).then_inc(s_tin1, 16)
            sync.dma_start(
                out=inp32[:, ICW:LIN], in_=inp_ext[:, ICW:LIN]
            ).then_inc(s_din1, 16)

            # t2: SBUF flat -> DRAM -> [80,128] blocks
            sync.wait_ge(s_t2dr, NT2)
            sync.dma_start(out=t2_dram[:], in_=t2flat[:, :]).then_inc(s_t2dma, 16)
            sync.wait_ge(s_t2dma, 16)
            sync.dma_start(
                out=t2sb[0:NBLK, :],
                in_=t2_dram.ap().rearrange("(p f) -> p f", p=NBLK),
            ).then_inc(s_t2dma, 16)

            # Hx[64:81] -> htop[0:17]
            sync.wait_ge(s_hx, 1)
            sync.dma_start(out=htop[0:NQ, :], in_=hx[64 : 64 + NQ, :]).then_inc(
                s_hdma, 16
            )

            # P -> DRAM (dense), DRAM -> skewsb (stride-2177 diagonal read)
            sync.wait_ge(s_drain, 2)
            sync.dma_start(
                out=AP(pskew_dram, 0, [[PW, 128], [1, PW]]),
                in_=psb[:, :],
            ).then_inc(s_skew, 16)
            sync.wait_ge(s_skew, 16)
            sync.dma_start(
                out=skewsb[:, :],
                in_=AP(pskew_dram, 0, [[SKST, 128], [1, S]]),
            ).then_inc(s_skew, 16)

            # loss partial -> DRAM for AR
            sync.wait_ge(s_loss, 1)
            sync.dma_start(out=lossp_dram[:, :], in_=loss_sb[:, :]).then_inc(s_ar, 16)

            # AR result -> SBUF
            sync.wait_ge(s_cc, 1)
            sync.dma_start(out=lossr_sb[:, :], in_=lossr_dram[:, :]).then_inc(
                s_arb, 16
            )

            # row stats transpose: [17,2] -> [1,34] in one SBUF->SBUF DMA
            sync.wait_ge(s_post, 1)
            sync.dma_start(out=rst_sb[:, :], in_=rs_sb[:, :]).then_inc(s_rs, 16)

            # final output
            sync.wait_ge(s_out, 1)
            sync.dma_start(out=out_ext[:, :], in_=out_sb[:, :]).then_inc(s_out, 16)

        @block.gpsimd
        def _(gpsimd: bass.BassGpSimd):
            # warmup collective: absorbs ncfw/collective-stream init cost
            # while DMAs and compute run; nobody waits on its result.
            gpsimd.memset(ccw_sb[:, :], 0.0).then_inc(s_ccw2, 1)
            gpsimd.wait_ge(s_ccw2, 1)
            gpsimd.dma_start(out=ccw_in_dram[:], in_=ccw_sb[:, :]).then_inc(s_ccw, 16)
            gpsimd.wait_ge(s_ccw, 16)
            gpsimd.collective_compute(
                "AllReduce",
                mybir.AluOpType.add,
                replica_groups=[list(range(NCORES))],
                ins=[ccw_in_dram.ap().opt()],
                outs=[ccw_out_dram.ap().opt()],
            ).then_inc(s_ccw2, 1)

            # constants; sem edges between dependent ops (Q7 cores overlap)
            gpsimd.memset(maskb[:, :], 1.0).then_inc(s_const, 1)
            gpsimd.wait_ge(s_const, 1)
            # keep where (k - q) >= 0
            gpsimd.affine_select(
                out=maskb[:, :], in_=maskb[:, :],
                pattern=[[-1, NQ]], compare_op=mybir.AluOpType.is_ge,
                fill=0.0, base=0, channel_multiplier=1,
            ).then_inc(s_const, 1)
            gpsimd.wait_ge(s_const, 2)
            # keep where (63 + q - k) >= 0
            gpsimd.affine_select(
                out=maskb[:, :], in_=maskb[:, :],
                pattern=[[1, NQ]], compare_op=mybir.AluOpType.is_ge,
                fill=0.0, base=63, channel_multiplier=-1,
            ).then_inc(s_const, 1)
            gpsimd.memset(ones17[:, :], 1.0).then_inc(s_const, 1)
            gpsimd.memset(onecol[:, :], 1.0).then_inc(s_const, 1)
            gpsimd.memset(onehot[:, :], 0.0).then_inc(s_const, 1)
            gpsimd.wait_ge(s_const, 6)
            gpsimd.memset(onehot[:, NQ - 1 : NQ], 1.0).then_inc(s_const, 1)
            gpsimd.iota(
                iota_i[0:NQ, :], pattern=[[1, 128]], base=0, channel_multiplier=128
            ).then_inc(s_const, 1)
            gpsimd.memset(maskadd[:, :], 0.0).then_inc(s_const, 1)
            gpsimd.wait_ge(s_const, 9)
            # keep 0 where (2048 - 128q - r) >= 0, else big
            gpsimd.affine_select(
                out=maskadd[:, :], in_=maskadd[:, :],
                pattern=[[-1, 128]], compare_op=mybir.AluOpType.is_ge,
                fill=1.0e30, base=S - 1, channel_multiplier=-128,
            ).then_inc(s_const, 1)

            # the collective
            gpsimd.wait_ge(s_ar, 16)
            gpsimd.collective_compute(
                "AllReduce",
                mybir.AluOpType.add,
                replica_groups=[list(range(NCORES))],
                ins=[lossp_dram.ap().opt()],
                outs=[lossr_dram.ap().opt()],
            ).then_inc(s_cc, 1)

        @block.vector
        def _(vector: bass.BassEngine):
            # every DVE op in the sequential sections incs s_v and waits on
            # the previous count, giving the race detector explicit edges.
            vcnt = [0]

            def vstep(mk, *waits):
                for sem, val in waits:
                    vector.wait_ge(sem, val)
                if vcnt[0] > 0:
                    vector.wait_ge(s_v, vcnt[0])
                mk().then_inc(s_v, 1)
                vcnt[0] += 1

            # chunked casts; input scaled by -2 during cast.  First chunks
            # of both tensors first so the PE can start early.
            vstep(lambda: vector.tensor_scalar(
                out=inpbf[:, 0:ICW], in0=inp32[:, 0:ICW], scalar1=-2.0,
                scalar2=None, op0=mybir.AluOpType.mult,
            ), (s_din, 16))
            vector.wait_ge(s_v, vcnt[0])
            vector.engine_nop().then_inc(s_cast, 1)
            vstep(lambda: vector.tensor_copy(tgtbf[:, 0:TCW], tgt32[:, 0:TCW]),
                  (s_tin, 16))
            vector.wait_ge(s_v, vcnt[0])
            vector.engine_nop().then_inc(s_tcast, 1)
            vstep(lambda: vector.tensor_scalar(
                out=inpbf[:, ICW:LIN], in0=inp32[:, ICW:LIN], scalar1=-2.0,
                scalar2=None, op0=mybir.AluOpType.mult,
            ), (s_din1, 16))
            vector.wait_ge(s_v, vcnt[0])
            vector.engine_nop().then_inc(s_cast, 1)
            tsems = [s_tin, s_tin1, s_tin2, s_tin3]
            for c in range(1, TCH):
                vstep(lambda c=c: vector.tensor_copy(
                    tgtbf[:, c * TCW : (c + 1) * TCW],
                    tgt32[:, c * TCW : (c + 1) * TCW],
                ), (tsems[c], 16))
                vector.wait_ge(s_v, vcnt[0])
                vector.engine_nop().then_inc(s_tcast, 1)

            # iota cast (after gpsimd consts)
            vstep(lambda: vector.tensor_copy(iota_f[:, :], iota_i[:, :]),
                  (s_const, 10))

            # colsum1 = per-bc sum of inp^2 (from ACT chunk accums)
            vstep(lambda: vector.reduce_sum(
                colsum1[:, :], sqacc[:, 0:4], axis=mybir.AxisListType.X),
                (s_isq, 4))
            vstep(lambda: vector.tensor_copy(colsum_bf[:, :], colsum1[:, :]))
            vector.wait_ge(s_v, vcnt[0])
            vector.engine_nop().then_inc(s_B, 1)

            # winsum path: inclusive prefix over 128-wide blocks
            vector.wait_ge(s_t2dma, 32)
            src, dst = t2sb, t2sb2
            for sh in (1, 2, 4, 8, 16, 32, 64):
                vstep(lambda src=src, dst=dst, sh=sh: vector.tensor_copy(
                    dst[0:NBLK, 0:sh], src[0:NBLK, 0:sh]))
                vstep(lambda src=src, dst=dst, sh=sh: vector.tensor_tensor(
                    out=dst[0:NBLK, sh:128],
                    in0=src[0:NBLK, sh:128],
                    in1=src[0:NBLK, 0 : 128 - sh],
                    op=mybir.AluOpType.add,
                ))
                src, dst = dst, src
            incl = src  # inclusive prefix lands here after 7 swaps
            # exclusive prefix hx (rows 64..80 zeroed first: row 80 stays 0
            # for the masked q=16 tail)
            vstep(lambda: vector.memset(hx[64 : NBLK + 1, :], 0.0))
            vstep(lambda: vector.memset(hx[0:NBLK, 0:1], 0.0))
            vstep(lambda: vector.tensor_copy(hx[0:NBLK, 1:128], incl[0:NBLK, 0:127]))
            vstep(lambda: vector.tensor_copy(b_bf[0:NBLK, :], incl[0:NBLK, 127:128]))
            vector.wait_ge(s_v, vcnt[0])
            vector.engine_nop().then_inc(s_hx, 1)  # hx ready (sync DMAs htop)
            vector.engine_nop().then_inc(s_B, 1)   # B(bf16) ready for CW mm

            # loss_sb = htop - hx[0:17]
            vstep(lambda: vector.tensor_tensor(
                out=loss_sb[:, :], in0=htop[:, :], in1=hx[0:NQ, :],
                op=mybir.AluOpType.subtract,
            ), (s_hdma, 16))

            # P drain part 1 (banks 0-1) -- must precede the skew-sum wait
            vstep(lambda: vector.tensor_copy(psb[:, 0:1024], pps[:, 0:1024]),
                  (s_pe, 1))
            vector.wait_ge(s_v, vcnt[0])
            vector.engine_nop().then_inc(s_drain, 1)

            # += CW (per-partition scalar)
            vstep(lambda: vector.tensor_scalar(
                out=loss_sb[:, :], in0=loss_sb[:, :], scalar1=cw_sb[0:NQ, 0:1],
                scalar2=None, op0=mybir.AluOpType.add,
            ), (s_cwdr, 1))
            # += (-2 corr) from skew-sum psum
            vstep(lambda: vector.tensor_tensor(
                out=loss_sb[:, :], in0=loss_sb[:, :],
                in1=miscps[0:NQ, 128:256],
                op=mybir.AluOpType.add,
            ), (s_smm, 1))
            # += mask (pre-AR; 8x-summed mask still dominates)
            vstep(lambda: vector.tensor_tensor(
                out=loss_sb[:, :], in0=loss_sb[:, :], in1=maskadd[:, :],
                op=mybir.AluOpType.add,
            ), (s_const, 10))
            vector.wait_ge(s_v, vcnt[0])
            vector.engine_nop().then_inc(s_loss, 1)

            # post-AR argmin chain
            vstep(lambda: vector.tensor_reduce(
                out=rs_sb[:, 0:1], in_=lossr_sb[:, :],
                axis=mybir.AxisListType.X, op=mybir.AluOpType.min,
            ), (s_arb, 16))
            vstep(lambda: vector.tensor_scalar(
                out=u_sb[:, :], in0=lossr_sb[:, :], scalar1=rs_sb[0:NQ, 0:1],
                scalar2=65536.0, op0=mybir.AluOpType.subtract,
                op1=mybir.AluOpType.mult,
            ))
            vstep(lambda: vector.tensor_tensor(
                out=u_sb[:, :], in0=u_sb[:, :], in1=iota_f[:, :],
                op=mybir.AluOpType.add,
            ))
            vstep(lambda: vector.tensor_reduce(
                out=rs_sb[:, 1:2], in_=u_sb[:, :],
                axis=mybir.AxisListType.X, op=mybir.AluOpType.min,
            ))
            vector.wait_ge(s_v, vcnt[0])
            vector.engine_nop().then_inc(s_post, 1)

            # after transpose-DMA: global min + argmin
            rmin_v = AP(rst_sb, 0, [[2 * NQ, 1], [2, NQ]])
            ridx_v = AP(rst_sb, 1, [[2 * NQ, 1], [2, NQ]])
            vstep(lambda: vector.tensor_reduce(
                out=mm_sb[:, :], in_=rmin_v,
                axis=mybir.AxisListType.X, op=mybir.AluOpType.min,
            ), (s_rs, 16))
            vstep(lambda: vector.tensor_scalar(
                out=um_sb[:, :], in0=rmin_v, scalar1=mm_sb[0:1, 0:1],
                scalar2=65536.0, op0=mybir.AluOpType.subtract,
                op1=mybir.AluOpType.mult,
            ))
            vstep(lambda: vector.tensor_tensor(
                out=um_sb[:, :], in0=um_sb[:, :], in1=ridx_v,
                op=mybir.AluOpType.add,
            ))
            vstep(lambda: vector.tensor_reduce(
                out=out_sb[:, 1:2], in_=um_sb[:, :],
                axis=mybir.AxisListType.X, op=mybir.AluOpType.min,
            ))
            vstep(lambda: vector.tensor_scalar(
                out=out_sb[:, 0:1], in0=mm_sb[:, :], scalar1=1.0 / NTOT,
                scalar2=None, op0=mybir.AluOpType.mult,
            ))
            vector.wait_ge(s_v, vcnt[0])
            vector.engine_nop().then_inc(s_out, 1)

        @block.scalar
        def _(scalar: bass.BassEngine):
            # issue the second half of the input loads from the ACT HWDGE
            # ring so the two physical rings split the load
            tsems = [s_tin, s_tin1, s_tin2, s_tin3]
            scalar.dma_start(
                out=tgt32[:, 2 * TCW : 3 * TCW], in_=tgt_ext[:, 2 * TCW : 3 * TCW]
            ).then_inc(s_tin2, 16)
            scalar.dma_start(
                out=tgt32[:, 3 * TCW : 4 * TCW], in_=tgt_ext[:, 3 * TCW : 4 * TCW]
            ).then_inc(s_tin3, 16)

            # target squared chunks as they arrive (t2 matmuls consume these)
            for c in range(TCH):
                scalar.wait_ge(tsems[c], 16)
                if c > 0:
                    scalar.wait_ge(s_sq, c)
                scalar.activation(
                    out=sqbf[:, c * TCW : (c + 1) * TCW],
                    in_=tgt32[:, c * TCW : (c + 1) * TCW],
                    func=mybir.ActivationFunctionType.Square,
                ).then_inc(s_sq, 1)

            # inp^2 row sums in 4 chunks (square + accumulate)
            scalar.wait_ge(s_din, 16)
            for i in range(4):
                if i == 2:
                    scalar.wait_ge(s_din1, 16)
                if i > 0:
                    scalar.wait_ge(s_isq, i)
                scalar.activation(
                    out=sqscr[:, :],
                    in_=inp32[:, i * 2048 : (i + 1) * 2048],
                    func=mybir.ActivationFunctionType.Square,
                    accum_out=sqacc[:, i : i + 1],
                ).then_inc(s_isq, 1)

            # drain t2 slabs PSUM -> t2flat (t2 matmuls run late, a>=40)
            for k in range(NT2):
                scalar.wait_ge(s_t2mm, k + 1)
                bank = (t2ps, t2ps2, miscps)[k % 3]
                scalar.copy(
                    t2flat[0:1, 512 * k : 512 * (k + 1)], bank[0:1, :]
                ).then_inc(s_t2dr, 1)

            # drain CW psum -> cw_sb
            scalar.wait_ge(s_cwmm, 1)
            scalar.copy(cw_sb[0:NQ, 0:1], miscps[0:NQ, 0:1]).then_inc(s_cwdr, 1)

            # P drain part 2 (banks 2-4)
            scalar.wait_ge(s_pe, 1)
            scalar.copy(psb[:, 1024:PW], pps[:, 1024:PW]).then_inc(s_drain, 1)

        @block.tensor
        def _(tensor: bass.BassEngine):
            t2k = 0
            icast = 1   # inpbf chunks available so far
            tcast = 1   # tgtbf chunks available so far
            tensor.wait_ge(s_const, 10)
            tensor.wait_ge(s_cast, 1)
            tensor.wait_ge(s_tcast, 1)
            for a in range(AW):
                if 128 * a + 128 > icast * ICW:
                    icast += 1
                    tensor.wait_ge(s_cast, icast)
                while 128 * a + PW > tcast * TCW:
                    tcast += 1
                    tensor.wait_ge(s_tcast, tcast)
                lhsT = inpbf[:, 128 * a : 128 * (a + 1)]
                base = 128 * a
                for j in range(4):
                    tensor.matmul(
                        out=pps[:, 512 * j : 512 * (j + 1)],
                        lhsT=lhsT,
                        rhs=tgtbf[:, base + 512 * j : base + 512 * (j + 1)],
                        start=(a == 0),
                        stop=(a == AW - 1),
                    )
                mm = tensor.matmul(
                    out=pps[:, 2048 : 2048 + 128],
                    lhsT=lhsT,
                    rhs=tgtbf[:, base + 2048 : base + PW],
                    start=(a == 0),
                    stop=(a == AW - 1),
                )
                if a == AW - 1:
                    mm.then_inc(s_pe, 1)
                # interleave one t2 matmul per iteration
                if T2MM_AT <= a < T2MM_AT + NT2:
                    tensor.wait_ge(s_sq, t2k // 5 + 1)
                    if t2k >= 3:
                        tensor.wait_ge(s_t2dr, t2k - 2)
                    bank = (t2ps, t2ps2, miscps)[t2k % 3]
                    tensor.matmul(
                        out=bank[0:1, :],
                        lhsT=onecol[:, :],
                        rhs=sqbf[:, 512 * t2k : 512 * (t2k + 1)],
                        start=True,
                        stop=True,
                    ).then_inc(s_t2mm, 1)
                    t2k += 1

            # CW matmuls: cw[q] = sum_{j=q..q+63} B[j] + inp_sq
            tensor.wait_ge(s_t2dr, NT2)
            tensor.wait_ge(s_B, 2)
            tensor.matmul(
                out=miscps[0:NQ, 0:1],
                lhsT=maskb[0:NBLK, :],
                rhs=b_bf[0:NBLK, :],
                start=True, stop=False,
            )
            tensor.matmul(
                out=miscps[0:NQ, 0:1],
                lhsT=ones17[:, :],
                rhs=colsum_bf[:, :],
                start=False, stop=True,
            ).then_inc(s_cwmm, 1)

            # skew-sum matmuls: misc[q,128+r] = sum_m skew[m, 128q+r]
            tensor.wait_ge(s_skew, 32)
            tensor.wait_ge(s_cwdr, 1)
            for q in range(NQ):
                ncols = 128 if q < NQ - 1 else 1
                mm = tensor.matmul(
                    out=miscps[0:NQ, 128 : 128 + ncols],
                    lhsT=onehot[:, NQ - 1 - q : 2 * NQ - 1 - q],
                    rhs=skewsb[:, 128 * q : 128 * q + ncols],
                    start=(q == 0),
                    stop=(q == NQ - 1),
                )
                if q == NQ - 1:
                    mm.then_inc(s_smm, 1)

    return nc


_NC_CACHE = None


def _get_nc():
    global _NC_CACHE
    if _NC_CACHE is None:
        _NC_CACHE = build_bass()
    return _NC_CACHE


def make_in_maps(input, target):
    inp = np.ascontiguousarray(np.asarray(input, dtype=np.float32))
    tgt = np.ascontiguousarray(np.asarray(target, dtype=np.float32))
    per = B // NCORES
    in_maps = []
    for c in range(NCORES):
        in_maps.append(
            {
                "input": np.ascontiguousarray(
                    inp[c * per : (c + 1) * per].reshape(BC, LIN)
                ),
                "target": np.ascontiguousarray(
                    tgt[c * per : (c + 1) * per].reshape(BC, LTGT)
                ),
            }
        )
    return in_maps


LAST_RESULTS = None


def kernel(input, target, trace=False, **trace_kwargs):
    global LAST_RESULTS
    from concourse.bass_utils import run_bass_kernel_spmd

    nc = _get_nc()
    in_maps = make_in_maps(input, target)
    res = run_bass_kernel_spmd(
        nc, in_maps, core_ids=list(range(NCORES)), trace=trace, **trace_kwargs
    )
    LAST_RESULTS = res
    out = res.results[0]["out"]
    min_loss = np.float32(out[0, 0])
    min_index = np.int32(np.rint(out[0, 1]))
    return (min_loss, min_index)


if __name__ == "__main__":
    nc = build_bass()
    print("bass graph built OK")


# revision 35
# speedup vs baseline: 1.3527x; 1.0369x over previous
"""Adaptive MSE loss (min over shifts) on 8 TRN2 NeuronCores.

Full inputs: input [16,64,8192] f32, target [16,64,10240] f32.
Data-parallel over batch B: 2 batches/core -> bc=128 rows on partitions.

Per core:
  P[m,u]    = sum_a sum_bc (-2*inp_bf)[bc,128a+m] * tgt_bf[bc,128a+u]   (PE, bf16)
  -2corr[s] = sum_m P[m, m+s]          (dense DRAM write + stride-2177 read skew,
                                        then one-hot fp32 matmuls -> [17,128])
  winsum[s] = CW[q] - Hx[q,r] + Hx[q+64,r]   (s=128q+r; block prefix sums of t2)
  t2[u]     = sum_bc tgt[bc,u]^2       (ones-matmul, bf16)
  inp_sq    = sum inp^2                (ACT square+accum, folded into CW matmul)
  loss*n    = inp_sq + winsum - 2corr  -> AllReduce [17,128] -> argmin on-device.

Inputs are loaded in chunks with casts/squares pipelined per chunk so the
PE starts ~15us in; a tiny warmup collective at t=0 absorbs ncfw init.
"""

import sys
import numpy as np

sys.path.insert(0, "/opt/trn_rl_repo")

from concourse import bass, mybir  # noqa: E402
from concourse.ap import AP  # noqa: E402

F32 = mybir.dt.float32
BF16 = mybir.dt.bfloat16
I32 = mybir.dt.int32

B, C, LIN, LTGT = 16, 64, 8192, 10240
NCORES = 8
BC = (B // NCORES) * C            # 128 rows per core
S = LTGT - LIN + 1                # 2049 shifts
AW = LIN // 128                   # 64 contraction chunks
PW = 2176                         # P width (17*128)
NQ = 17                           # shift blocks (s = 128q+r)
NBLK = LTGT // 128                # 80 t2 blocks
NTOT = float(B * C * LIN)         # 8388608
SKST = PW + 1                     # 2177: skew read row stride (write is dense)

ICH = 2                           # input DMA chunks (4096 cols each)
TCH = 4                           # target DMA chunks (2560 cols each)
ICW = LIN // ICH
TCW = LTGT // TCH
T2MM_AT = 24                      # corr-loop index where t2 matmuls interleave
NT2 = LTGT // 512                 # 20 t2 slabs


def build_bass():
    nc = bass.Bass(num_devices=NCORES)

    inp_ext = nc.declare_dram_parameter("input", [BC, LIN], F32, isOutput=False)
    tgt_ext = nc.declare_dram_parameter("target", [BC, LTGT], F32, isOutput=False)
    out_ext = nc.declare_dram_parameter("out", [1, 2], F32, isOutput=True)

    t2_dram = nc.dram_tensor("t2_dram", [NBLK * 128], F32)
    pskew_dram = nc.dram_tensor("pskew_dram", [128 * PW], BF16)
    lossp_dram = nc.dram_tensor("lossp_dram", [NQ, 128], F32)
    lossr_dram = nc.dram_tensor("lossr_dram", [NQ, 128], F32, addr_space="Shared")
    ccw_in_dram = nc.dram_tensor("ccw_in_dram", [128], F32)
    ccw_out_dram = nc.dram_tensor("ccw_out_dram", [128], F32, addr_space="Shared")

    # SBUF
    inp32 = nc.alloc_sbuf_tensor("inp32", [BC, LIN], F32)
    tgt32 = nc.alloc_sbuf_tensor("tgt32", [BC, LTGT], F32)
    inpbf = nc.alloc_sbuf_tensor("inpbf", [BC, LIN], BF16)
    tgtbf = nc.alloc_sbuf_tensor("tgtbf", [BC, LTGT], BF16)
    sqbf = nc.alloc_sbuf_tensor("sqbf", [BC, LTGT], BF16)
    sqscr = nc.alloc_sbuf_tensor("sqscr", [BC, 2048], BF16)
    sqacc = nc.alloc_sbuf_tensor("sqacc", [BC, 8], F32)
    colsum1 = nc.alloc_sbuf_tensor("colsum1", [BC, 1], F32)
    t2flat = nc.alloc_sbuf_tensor("t2flat", [1, LTGT], F32)
    psb_off = nc.sbuf_base
    psb = nc.alloc_sbuf_tensor("psb", [128, PW], BF16)
    # skewsb aliases psb: psb is dead once its DMA-out completes, and the
    # skew-read DMA that fills skewsb is sem-ordered after that DMA-out.
    skewsb = nc.alloc_sbuf_tensor_at("skewsb", [128, S], BF16, offset=psb_off)
    t2sb = nc.alloc_sbuf_tensor("t2sb", [128, 128], F32)
    t2sb2 = nc.alloc_sbuf_tensor("t2sb2", [128, 128], F32)
    hx = nc.alloc_sbuf_tensor("hx", [128, 128], F32)
    htop = nc.alloc_sbuf_tensor("htop", [NQ, 128], F32)
    loss_sb = nc.alloc_sbuf_tensor("loss_sb", [NQ, 128], F32)
    lossr_sb = nc.alloc_sbuf_tensor("lossr_sb", [NQ, 128], F32)
    cw_sb = nc.alloc_sbuf_tensor("cw_sb", [NQ, 1], F32)
    b_bf = nc.alloc_sbuf_tensor("b_bf", [128, 1], BF16)
    colsum_bf = nc.alloc_sbuf_tensor("colsum_bf", [128, 1], BF16)
    iota_i = nc.alloc_sbuf_tensor("iota_i", [NQ, 128], I32)
    iota_f = nc.alloc_sbuf_tensor("iota_f", [NQ, 128], F32)
    maskb = nc.alloc_sbuf_tensor("maskb", [128, NQ], BF16)
    ones17 = nc.alloc_sbuf_tensor("ones17", [128, NQ], BF16)
    onecol = nc.alloc_sbuf_tensor("onecol", [128, 1], BF16)
    onehot = nc.alloc_sbuf_tensor("onehot", [128, 2 * NQ - 1], BF16)
    maskadd = nc.alloc_sbuf_tensor("maskadd", [NQ, 128], F32)
    rs_sb = nc.alloc_sbuf_tensor("rs_sb", [NQ, 2], F32)
    rst_sb = nc.alloc_sbuf_tensor("rst_sb", [1, 2 * NQ], F32)
    u_sb = nc.alloc_sbuf_tensor("u_sb", [NQ, 128], F32)
    um_sb = nc.alloc_sbuf_tensor("um_sb", [1, NQ], F32)
    mm_sb = nc.alloc_sbuf_tensor("mm_sb", [1, 1], F32)
    out_sb = nc.alloc_sbuf_tensor("out_sb", [1, 2], F32)
    ccw_sb = nc.alloc_sbuf_tensor("ccw_sb", [128, 1], F32)

    # PSUM: bank-aligned layout (8 banks x 512 f32)
    pps = nc.alloc_psum_tensor("pps", [128, 2560], F32)       # banks 0-4
    t2ps = nc.alloc_psum_tensor("t2ps", [128, 512], F32)      # bank 5
    t2ps2 = nc.alloc_psum_tensor("t2ps2", [128, 512], F32)    # bank 6
    miscps = nc.alloc_psum_tensor("miscps", [128, 512], F32)  # bank 7

    from contextlib import ExitStack

    with ExitStack() as stack:
        block = stack.enter_context(nc.Block())
        sem_names = [
            "s_din", "s_din1", "s_tin", "s_tin1", "s_tin2", "s_tin3",
            "s_cast", "s_tcast", "s_sq", "s_isq", "s_t2mm",
            "s_t2dr", "s_t2dma", "s_const", "s_B", "s_hx", "s_hdma", "s_cwmm",
            "s_cwdr", "s_pe", "s_drain", "s_skew", "s_smm", "s_loss", "s_ar",
            "s_cc", "s_arb", "s_post", "s_rs", "s_out", "s_v", "s_ccw", "s_ccw2",
        ]
        sems = {n: stack.enter_context(nc.semaphore(n)) for n in sem_names}
        (
            s_din, s_din1, s_tin, s_tin1, s_tin2, s_tin3,
            s_cast, s_tcast, s_sq, s_isq, s_t2mm, s_t2dr,
            s_t2dma, s_const, s_B, s_hx, s_hdma, s_cwmm, s_cwdr, s_pe,
            s_drain, s_skew, s_smm, s_loss, s_ar, s_cc, s_arb, s_post,
            s_rs, s_out, s_v, s_ccw, s_ccw2,
        ) = (sems[n] for n in sem_names)

        @block.sync
        def _(sync: bass.BassEngine):
            # input loads split across the two HWDGE rings; this ring
            # carries tgt c0, inp c0, tgt c1, inp c1 (earliest-needed);
            # the ACT ring (whose boilerplate delays issue ~10us) carries
            # the late chunks tgt c2, tgt c3.
            sync.dma_start(
                out=tgt32[:, 0:TCW], in_=tgt_ext[:, 0:TCW]
            ).then_inc(s_tin, 16)
            sync.dma_start(
                out=inp32[:, 0:ICW], in_=inp_ext[:, 0:ICW]
            ).then_inc(s_din, 16)
            sync.wait_ge(s_tin, 16)
            sync.dma_start(
                out=tgt32[:, TCW : 2 * TCW], in_=tgt_ext[:, TCW : 2 * TCW]
            ).then_inc(s_tin1, 16)
            sync.dma_start(
                out=inp32[:, ICW:LIN], in_=inp_ext[:, ICW:LIN]
            ).then_inc(s_din1, 16)

            # t2: SBUF flat -> DRAM -> [80,128] blocks
            sync.wait_ge(s_t2dr, NT2)
            sync.dma_start(out=t2_dram[:], in_=t2flat[:, :]).then_inc(s_t2dma, 16)
            sync.wait_ge(s_t2dma, 16)
            sync.dma_start(
                out=t2sb[0:NBLK, :],
                in_=t2_dram.ap().rearrange("(p f) -> p f", p=NBLK),
            ).then_inc(s_t2dma, 16)

            # Hx[64:81] -> htop[0:17]
            sync.wait_ge(s_hx, 1)
            sync.dma_start(out=htop[0:NQ, :], in_=hx[64 : 64 + NQ, :]).then_inc(
                s_hdma, 16
            )

            # P -> DRAM (dense), DRAM -> skewsb (stride-2177 diagonal read)
            sync.wait_ge(s_drain, 2)
            sync.dma_start(
                out=AP(pskew_dram, 0, [[PW, 128], [1, PW]]),
                in_=psb[:, :],
            ).then_inc(s_skew, 16)
            sync.wait_ge(s_skew, 16)
            sync.dma_start(
                out=skewsb[:, :],
                in_=AP(pskew_dram, 0, [[SKST, 128], [1, S]]),
            ).then_inc(s_skew, 16)

            # loss partial -> DRAM for AR
            sync.wait_ge(s_loss, 1)
            sync.dma_start(out=lossp_dram[:, :], in_=loss_sb[:, :]).then_inc(s_ar, 16)

            # AR result -> SBUF
            sync.wait_ge(s_cc, 1)
            sync.dma_start(out=lossr_sb[:, :], in_=lossr_dram[:, :]).then_inc(
                s_arb, 16
            )

            # row stats transpose: [17,2] -> [1,34] in one SBUF->SBUF DMA
            sync.wait_ge(s_post, 1)
            sync.dma_start(out=rst_sb[:, :], in_=rs_sb[:, :]).then_inc(s_rs, 16)

            # final output
            sync.wait_ge(s_out, 1)
            sync.dma_start(out=out_ext[:, :], in_=out_sb[:, :]).then_inc(s_out, 16)

        @block.gpsimd
        def _(gpsimd: bass.BassGpSimd):
            # warmup collective: absorbs ncfw/collective-stream init cost
            # while DMAs and compute run; nobody waits on its result.
            gpsimd.memset(ccw_sb[:, :], 0.0).then_inc(s_ccw2, 1)
            gpsimd.wait_ge(s_ccw2, 1)
            gpsimd.dma_start(out=ccw_in_dram[:], in_=ccw_sb[:, :]).then_inc(s_ccw, 16)
            gpsimd.wait_ge(s_ccw, 16)
            gpsimd.collective_compute(
                "AllReduce",
                mybir.AluOpType.add,
                replica_groups=[list(range(NCORES))],
                ins=[ccw_in_dram.ap().opt()],
                outs=[ccw_out_dram.ap().opt()],
            ).then_inc(s_ccw2, 1)

            # constants; sem edges between dependent ops (Q7 cores overlap)
            gpsimd.memset(maskb[:, :], 1.0).then_inc(s_const, 1)
            gpsimd.wait_ge(s_const, 1)
            # keep where (k - q) >= 0
            gpsimd.affine_select(
                out=maskb[:, :], in_=maskb[:, :],
                pattern=[[-1, NQ]], compare_op=mybir.AluOpType.is_ge,
                fill=0.0, base=0, channel_multiplier=1,
            ).then_inc(s_const, 1)
            gpsimd.wait_ge(s_const, 2)
            # keep where (63 + q - k) >= 0
            gpsimd.affine_select(
                out=maskb[:, :], in_=maskb[:, :],
                pattern=[[1, NQ]], compare_op=mybir.AluOpType.is_ge,
                fill=0.0, base=63, channel_multiplier=-1,
            ).then_inc(s_const, 1)
            gpsimd.memset(ones17[:, :], 1.0).then_inc(s_const, 1)
            gpsimd.memset(onecol[:, :], 1.0).then_inc(s_const, 1)
            gpsimd.memset(onehot[:, :], 0.0).then_inc(s_const, 1)
            gpsimd.wait_ge(s_const, 6)
            gpsimd.memset(onehot[:, NQ - 1 : NQ], 1.0).then_inc(s_const, 1)
            gpsimd.iota(
                iota_i[0:NQ, :], pattern=[[1, 128]], base=0, channel_multiplier=128
            ).then_inc(s_const, 1)
            gpsimd.memset(maskadd[:, :], 0.0).then_inc(s_const, 1)
            gpsimd.wait_ge(s_const, 9)
            # keep 0 where (2048 - 128q - r) >= 0, else big
            gpsimd.affine_select(
                out=maskadd[:, :], in_=maskadd[:, :],
                pattern=[[-1, 128]], compare_op=mybir.AluOpType.is_ge,
                fill=1.0e30, base=S - 1, channel_multiplier=-128,
            ).then_inc(s_const, 1)

            # the collective
            gpsimd.wait_ge(s_ar, 16)
            gpsimd.collective_compute(
                "AllReduce",
                mybir.AluOpType.add,
                replica_groups=[list(range(NCORES))],
                ins=[lossp_dram.ap().opt()],
                outs=[lossr_dram.ap().opt()],
            ).then_inc(s_cc, 1)

        @block.vector
        def _(vector: bass.BassEngine):
            # every DVE op in the sequential sections incs s_v and waits on
            # the previous count, giving the race detector explicit edges.
            vcnt = [0]

            def vstep(mk, *waits):
                for sem, val in waits:
                    vector.wait_ge(sem, val)
                if vcnt[0] > 0:
                    vector.wait_ge(s_v, vcnt[0])
                mk().then_inc(s_v, 1)
                vcnt[0] += 1

            # chunked casts; input scaled by -2 during cast.  First chunks
            # of both tensors first so the PE can start early.
            vstep(lambda: vector.tensor_scalar(
                out=inpbf[:, 0:ICW], in0=inp32[:, 0:ICW], scalar1=-2.0,
                scalar2=None, op0=mybir.AluOpType.mult,
            ), (s_din, 16))
            vector.wait_ge(s_v, vcnt[0])
            vector.engine_nop().then_inc(s_cast, 1)
            vstep(lambda: vector.tensor_copy(tgtbf[:, 0:TCW], tgt32[:, 0:TCW]),
                  (s_tin, 16))
            vector.wait_ge(s_v, vcnt[0])
            vector.engine_nop().then_inc(s_tcast, 1)
            vstep(lambda: vector.tensor_scalar(
                out=inpbf[:, ICW:LIN], in0=inp32[:, ICW:LIN], scalar1=-2.0,
                scalar2=None, op0=mybir.AluOpType.mult,
            ), (s_din1, 16))
            vector.wait_ge(s_v, vcnt[0])
            vector.engine_nop().then_inc(s_cast, 1)
            tsems = [s_tin, s_tin1, s_tin2, s_tin3]
            for c in range(1, TCH):
                vstep(lambda c=c: vector.tensor_copy(
                    tgtbf[:, c * TCW : (c + 1) * TCW],
                    tgt32[:, c * TCW : (c + 1) * TCW],
                ), (tsems[c], 16))
                vector.wait_ge(s_v, vcnt[0])
                vector.engine_nop().then_inc(s_tcast, 1)

            # iota cast (after gpsimd consts)
            vstep(lambda: vector.tensor_copy(iota_f[:, :], iota_i[:, :]),
                  (s_const, 10))

            # colsum1 = per-bc sum of inp^2 (from ACT chunk accums)
            vstep(lambda: vector.reduce_sum(
                colsum1[:, :], sqacc[:, 0:4], axis=mybir.AxisListType.X),
                (s_isq, 4))
            vstep(lambda: vector.tensor_copy(colsum_bf[:, :], colsum1[:, :]))
            vector.wait_ge(s_v, vcnt[0])
            vector.engine_nop().then_inc(s_B, 1)

            # winsum path: inclusive prefix over 128-wide blocks
            vector.wait_ge(s_t2dma, 32)
            src, dst = t2sb, t2sb2
            for sh in (1, 2, 4, 8, 16, 32, 64):
                vstep(lambda src=src, dst=dst, sh=sh: vector.tensor_copy(
                    dst[0:NBLK, 0:sh], src[0:NBLK, 0:sh]))
                vstep(lambda src=src, dst=dst, sh=sh: vector.tensor_tensor(
                    out=dst[0:NBLK, sh:128],
                    in0=src[0:NBLK, sh:128],
                    in1=src[0:NBLK, 0 : 128 - sh],
                    op=mybir.AluOpType.add,
                ))
                src, dst = dst, src
            incl = src  # inclusive prefix lands here after 7 swaps
            # exclusive prefix hx (rows 64..80 zeroed first: row 80 stays 0
            # for the masked q=16 tail)
            vstep(lambda: vector.memset(hx[64 : NBLK + 1, :], 0.0))
            vstep(lambda: vector.memset(hx[0:NBLK, 0:1], 0.0))
            vstep(lambda: vector.tensor_copy(hx[0:NBLK, 1:128], incl[0:NBLK, 0:127]))
            vstep(lambda: vector.tensor_copy(b_bf[0:NBLK, :], incl[0:NBLK, 127:128]))
            vector.wait_ge(s_v, vcnt[0])
            vector.engine_nop().then_inc(s_hx, 1)  # hx ready (sync DMAs htop)
            vector.engine_nop().then_inc(s_B, 1)   # B(bf16) ready for CW mm

            # loss_sb = htop - hx[0:17]
            vstep(lambda: vector.tensor_tensor(
                out=loss_sb[:, :], in0=htop[:, :], in1=hx[0:NQ, :],
                op=mybir.AluOpType.subtract,
            ), (s_hdma, 16))

            # P drain part 1 (banks 0-1) -- must precede the skew-sum wait
            vstep(lambda: vector.tensor_copy(psb[:, 0:1024], pps[:, 0:1024]),
                  (s_pe, 1))
            vector.wait_ge(s_v, vcnt[0])
            vector.engine_nop().then_inc(s_drain, 1)

            # += CW (per-partition scalar)
            vstep(lambda: vector.tensor_scalar(
                out=loss_sb[:, :], in0=loss_sb[:, :], scalar1=cw_sb[0:NQ, 0:1],
                scalar2=None, op0=mybir.AluOpType.add,
            ), (s_cwdr, 1))
            # += (-2 corr) from skew-sum psum
            vstep(lambda: vector.tensor_tensor(
                out=loss_sb[:, :], in0=loss_sb[:, :],
                in1=miscps[0:NQ, 128:256],
                op=mybir.AluOpType.add,
            ), (s_smm, 1))
            # += mask (pre-AR; 8x-summed mask still dominates)
            vstep(lambda: vector.tensor_tensor(
                out=loss_sb[:, :], in0=loss_sb[:, :], in1=maskadd[:, :],
                op=mybir.AluOpType.add,
            ), (s_const, 10))
            vector.wait_ge(s_v, vcnt[0])
            vector.engine_nop().then_inc(s_loss, 1)

            # post-AR argmin chain
            vstep(lambda: vector.tensor_reduce(
                out=rs_sb[:, 0:1], in_=lossr_sb[:, :],
                axis=mybir.AxisListType.X, op=mybir.AluOpType.min,
            ), (s_arb, 16))
            vstep(lambda: vector.tensor_scalar(
                out=u_sb[:, :], in0=lossr_sb[:, :], scalar1=rs_sb[0:NQ, 0:1],
                scalar2=65536.0, op0=mybir.AluOpType.subtract,
                op1=mybir.AluOpType.mult,
            ))
            vstep(lambda: vector.tensor_tensor(
                out=u_sb[:, :], in0=u_sb[:, :], in1=iota_f[:, :],
                op=mybir.AluOpType.add,
            ))
            vstep(lambda: vector.tensor_reduce(
                out=rs_sb[:, 1:2], in_=u_sb[:, :],
                axis=mybir.AxisListType.X, op=mybir.AluOpType.min,
            ))
            vector.wait_ge(s_v, vcnt[0])
            vector.engine_nop().then_inc(s_post, 1)

            # after transpose-DMA: global min + argmin
            rmin_v = AP(rst_sb, 0, [[2 * NQ, 1], [2, NQ]])
            ridx_v = AP(rst_sb, 1, [[2 * NQ, 1], [2, NQ]])
            vstep(lambda: vector.tensor_reduce(
                out=mm_sb[:, :], in_=rmin_v,
                axis=mybir.AxisListType.X, op=mybir.AluOpType.min,
            ), (s_rs, 16))
            vstep(lambda: vector.tensor_scalar(
                out=um_sb[:, :], in0=rmin_v, scalar1=mm_sb[0:1, 0:1],
                scalar2=65536.0, op0=mybir.AluOpType.subtract,
                op1=mybir.AluOpType.mult,
            ))
            vstep(lambda: vector.tensor_tensor(
                out=um_sb[:, :], in0=um_sb[:, :], in1=ridx_v,
                op=mybir.AluOpType.add,
            ))
            vstep(lambda: vector.tensor_reduce(
                out=out_sb[:, 1:2], in_=um_sb[:, :],
                axis=mybir.AxisListType.X, op=mybir.AluOpType.min,
            ))
            vstep(lambda: vector.tensor_scalar(
                out=out_sb[:, 0:1], in0=mm_sb[:, :], scalar1=1.0 / NTOT,
                scalar2=None, op0=mybir.AluOpType.mult,
            ))
            vector.wait_ge(s_v, vcnt[0])
            vector.engine_nop().then_inc(s_out, 1)

        @block.scalar
        def _(scalar: bass.BassEngine):
            # issue the second half of the input loads from the ACT HWDGE
            # ring so the two physical rings split the load
            tsems = [s_tin, s_tin1, s_tin2, s_tin3]
            scalar.dma_start(
                out=tgt32[:, 2 * TCW : 3 * TCW], in_=tgt_ext[:, 2 * TCW : 3 * TCW]
            ).then_inc(s_tin2, 16)
            scalar.dma_start(
                out=tgt32[:, 3 * TCW : 4 * TCW], in_=tgt_ext[:, 3 * TCW : 4 * TCW]
            ).then_inc(s_tin3, 16)

            # target squared chunks as they arrive (t2 matmuls consume these)
            for c in range(TCH):
                scalar.wait_ge(tsems[c], 16)
                if c > 0:
                    scalar.wait_ge(s_sq, c)
                scalar.activation(
                    out=sqbf[:, c * TCW : (c + 1) * TCW],
                    in_=tgt32[:, c * TCW : (c + 1) * TCW],
                    func=mybir.ActivationFunctionType.Square,
                ).then_inc(s_sq, 1)

            # inp^2 row sums in 4 chunks (square + accumulate)
            scalar.wait_ge(s_din, 16)
            for i in range(4):
                if i == 2:
                    scalar.wait_ge(s_din1, 16)
                if i > 0:
                    scalar.wait_ge(s_isq, i)
                scalar.activation(
                    out=sqscr[:, :],
                    in_=inp32[:, i * 2048 : (i + 1) * 2048],
                    func=mybir.ActivationFunctionType.Square,
                    accum_out=sqacc[:, i : i + 1],
                ).then_inc(s_isq, 1)

            # drain t2 slabs PSUM -> t2flat (t2 matmuls run late, a>=40)
            for k in range(NT2):
                scalar.wait_ge(s_t2mm, k + 1)
                bank = (t2ps, t2ps2, miscps)[k % 3]
                scalar.copy(
                    t2flat[0:1, 512 * k : 512 * (k + 1)], bank[0:1, :]
                ).then_inc(s_t2dr, 1)

            # drain CW psum -> cw_sb
            scalar.wait_ge(s_cwmm, 1)
            scalar.copy(cw_sb[0:NQ, 0:1], miscps[0:NQ, 0:1]).then_inc(s_cwdr, 1)

            # P drain part 2 (banks 2-4)
            scalar.wait_ge(s_pe, 1)
            scalar.copy(psb[:, 1024:PW], pps[:, 1024:PW]).then_inc(s_drain, 1)

        @block.tensor
        def _(tensor: bass.BassEngine):
            t2k = 0
            icast = 1   # inpbf chunks available so far
            tcast = 1   # tgtbf chunks available so far
            tensor.wait_ge(s_const, 10)
            tensor.wait_ge(s_cast, 1)
            tensor.wait_ge(s_tcast, 1)
            for a in range(AW):
                if 128 * a + 128 > icast * ICW:
                    icast += 1
                    tensor.wait_ge(s_cast, icast)
                while 128 * a + PW > tcast * TCW:
                    tcast += 1
                    tensor.wait_ge(s_tcast, tcast)
                lhsT = inpbf[:, 128 * a : 128 * (a + 1)]
                base = 128 * a
                for j in range(4):
                    tensor.matmul(
                        out=pps[:, 512 * j : 512 * (j + 1)],
                        lhsT=lhsT,
                        rhs=tgtbf[:, base + 512 * j : base + 512 * (j + 1)],
                        start=(a == 0),
                        stop=(a == AW - 1),
                    )
                mm = tensor.matmul(
                    out=pps[:, 2048 : 2048 + 128],
                    lhsT=lhsT,
                    rhs=tgtbf[:, base + 2048 : base + PW],
                    start=(a == 0),
                    stop=(a == AW - 1),
                )
                if a == AW - 1:
                    mm.then_inc(s_pe, 1)
                # interleave one t2 matmul per iteration
                if T2MM_AT <= a < T2MM_AT + NT2:
                    tensor.wait_ge(s_sq, t2k // 5 + 1)
                    if t2k >= 3:
                        tensor.wait_ge(s_t2dr, t2k - 2)
                    bank = (t2ps, t2ps2, miscps)[t2k % 3]
                    tensor.matmul(
                        out=bank[0:1, :],
                        lhsT=onecol[:, :],
                        rhs=sqbf[:, 512 * t2k : 512 * (t2k + 1)],
                        start=True,
                        stop=True,
                    ).then_inc(s_t2mm, 1)
                    t2k += 1

            # CW matmuls: cw[q] = sum_{j=q..q+63} B[j] + inp_sq
            tensor.wait_ge(s_t2dr, NT2)
            tensor.wait_ge(s_B, 2)
            tensor.matmul(
                out=miscps[0:NQ, 0:1],
                lhsT=maskb[0:NBLK, :],
                rhs=b_bf[0:NBLK, :],
                start=True, stop=False,
            )
            tensor.matmul(
                out=miscps[0:NQ, 0:1],
                lhsT=ones17[:, :],
                rhs=colsum_bf[:, :],
                start=False, stop=True,
            ).then_inc(s_cwmm, 1)

            # skew-sum matmuls: misc[q,128+r] = sum_m skew[m, 128q+r]
            tensor.wait_ge(s_skew, 32)
            tensor.wait_ge(s_cwdr, 1)
            for q in range(NQ):
                ncols = 128 if q < NQ - 1 else 1
                mm = tensor.matmul(
                    out=miscps[0:NQ, 128 : 128 + ncols],
                    lhsT=onehot[:, NQ - 1 - q : 2 * NQ - 1 - q],
                    rhs=skewsb[:, 128 * q : 128 * q + ncols],
                    start=(q == 0),
                    stop=(q == NQ - 1),
                )
                if q == NQ - 1:
                    mm.then_inc(s_smm, 1)

    return nc


_NC_CACHE = None


def _get_nc():
    global _NC_CACHE
    if _NC_CACHE is None:
        _NC_CACHE = build_bass()
    return _NC_CACHE


def make_in_maps(input, target):
    inp = np.ascontiguousarray(np.asarray(input, dtype=np.float32))
    tgt = np.ascontiguousarray(np.asarray(target, dtype=np.float32))
    per = B // NCORES
    in_maps = []
    for c in range(NCORES):
        in_maps.append(
            {
                "input": np.ascontiguousarray(
                    inp[c * per : (c + 1) * per].reshape(BC, LIN)
                ),
                "target": np.ascontiguousarray(
                    tgt[c * per : (c + 1) * per].reshape(BC, LTGT)
                ),
            }
        )
    return in_maps


LAST_RESULTS = None


def kernel(input, target, trace=False, **trace_kwargs):
    global LAST_RESULTS
    from concourse.bass_utils import run_bass_kernel_spmd

    nc = _get_nc()
    in_maps = make_in_maps(input, target)
    res = run_bass_kernel_spmd(
        nc, in_maps, core_ids=list(range(NCORES)), trace=trace, **trace_kwargs
    )
    LAST_RESULTS = res
    out = res.results[0]["out"]
    min_loss = np.float32(out[0, 0])
    min_index = np.int32(np.rint(out[0, 1]))
    return (min_loss, min_index)


if __name__ == "__main__":
    nc = build_bass()
    print("bass graph built OK")
